# revision 40
# baseline (speedup 1.0000x reference)
"""Multi-head causal attention (B=2, S=2048, D=1024, H=16) on 8 trn2 NeuronCores.

Strategy (tensor-parallel over heads, per the sharding hint):
  - Each core owns 2 heads (128 of 1024 hidden dims): W_q/W_k/W_v column-parallel.
  - Activations kept transposed ([dim, token]) end to end so every matmul
    contracts on the partition axis with zero on-device transposes of x.
  - Projections run per 1024-token pair of tiles (x loaded in 2MB chunks);
    each matmul streams 512 tokens (one fp32 PSUM bank); attention q-tiles
    are 512 wide. Diagonal-chunk score/PV matmuls stream only the unmasked
    query range.
  - scores^T = K^T.T @ Q^T per 128-key-chunk x 512-query-tile, two heads packed
    into disjoint PE row-groups (contraction is only dk=64).
  - softmax without max-subtraction (scores are O(1)); rowsum folded into the
    PV matmul via an augmented V [keys, 64+1] whose last column is ones.
  - exp only on the causal part of diagonal chunks; the rest of the P tile is
    zeroed, and only the 128-wide diagonal strip is tri-masked.
  - normalization: rowsum rows gathered to [128, 8] for one 128-lane DVE
    reciprocal, scattered back, broadcast via a PE outer-product; the finish
    (broadcast+multiply+ship) is deferred >= one full iteration so the PE
    never waits on the chain.
  - q-tiles processed batch-interleaved (b0j0, b1j0, b0j1, ...) and ctx
    re-sharded token-parallel with FOUR AllToAlls (one per half-batch); the
    gpsimd ring carries ONLY the collective triggers, so a busy CC engine
    can never stall compute; each a2a DRAM buffer has its own pool tag
    (shared-tag tiles alias one slot and serialize ships behind collectives).
  - out-projection (full W_o) per 128-token quarter at the tail, filling
    the PE while a2a(2)/(3) fly; only the last a2a + one quarter is exposed.
  - bf16 matmul inputs everywhere; PSUM accumulation stays fp32; the
    softmax reciprocal and the final output are bf16.

kernel(**inputs) takes the full unsharded inputs and returns the full output.
"""

import numpy as np
import ml_dtypes

import concourse.bass as bass
import concourse.mybir as mybir
import concourse.tile as tile
from concourse import bacc
from concourse.bass_utils import run_bass_kernel_spmd

B, S, D = 2, 2048, 1024
H, DK = 16, 64
NCORE = 8
T = B * S          # 4096 tokens
TT = 512           # attention q-tile width
PT = 1024          # projection pair width
NT = T // TT       # 8 token tiles
NP = T // PT       # 4 projection pairs
KC = 128           # key chunk
NJ = S // TT       # 4 q-tiles per batch
SCALE = 1.0 / np.sqrt(DK)

# batch-interleaved q-tile order; ORDER[i] = (b, j), its token tile is b*NJ+j
ORDER = [(0, 0), (1, 0), (0, 1), (1, 1), (0, 2), (1, 2), (0, 3), (1, 3)]
TILE_OF = [b * NJ + j for (b, j) in ORDER]
# a2a group of q-tile (b, j); groups pair tiles that finish adjacently so
# each a2a's inputs complete as early as possible: G0/G1 = j<2 per batch,
# G2 = both j=2 tiles (done by i5), G3 = both j=3 tiles (the tail pair)
A_OF = {(b, j): (b if j < 2 else j) for (b, j) in ORDER}
# dst slab base within a group: j<2 -> by j, j>=2 -> by batch
G0_OF = {(b, j): (4 * j if j < 2 else 4 * b) for (b, j) in ORDER}

f32 = mybir.dt.float32
bf16 = mybir.dt.bfloat16
EXP = mybir.ActivationFunctionType.Exp
MULT = mybir.AluOpType.mult
npbf = ml_dtypes.bfloat16


def build_program():
    nc = bacc.Bacc("TRN2", target_bir_lowering=False, debug=False,
                   num_devices=NCORE)

    # declaration order ~ host restage order: xTp0 + wT first (they gate the
    # first projection), xTp1-3 stream behind, woT last (tail-only)
    # restage follows declaration order; order by first consumption:
    # xTp0 + wT + bqkv gate the first projection, xTp2 is consumed at i=0,
    # ident/trimask at the first vtrans/diag chunk, xTp1 at i=2, xTp3 at
    # i=3, woT only at the tail
    # pairs 1-3 restage as o-halves with separate ready semaphores so the
    # projections can consume each half the moment it lands (the slowest
    # core's restage runs at ~65GB/s and otherwise paces the whole fleet)
    def xin(name, shape):
        return nc.dram_tensor(name, shape, bf16, kind="ExternalInput").ap()

    xTp_d = {0: [xin("xTp0", [128, 8, PT])]}
    wT_d = xin("wT", [128, 8, 3, 128])
    bqkv_d = nc.dram_tensor("bqkv", [128, 3], f32, kind="ExternalInput").ap()
    xTp_d[2] = [xin("xTp2a", [128, 4, PT]), xin("xTp2b", [128, 4, PT])]
    ident_d = xin("ident", [128, 128])
    trimask_d = xin("trimask", [128, 128])
    xTp_d[1] = [xin("xTp1a", [128, 4, PT]), xin("xTp1b", [128, 4, PT])]
    xTp_d[3] = [xin("xTp3a", [128, 4, PT]), xin("xTp3b", [128, 4, PT])]
    bo_d = nc.dram_tensor("bo", [1, 1024], f32, kind="ExternalInput").ap()
    woT_d = xin("woT", [128, 8, 1024])
    # outT[k] = this core's 128-token slice of a2a group k (see GROUP_TOK)
    outT_d = nc.dram_tensor("outT", [4, 128, 1024], bf16, kind="ExternalOutput").ap()

    with tile.TileContext(nc) as tc:
        with (
            tc.tile_pool(name="const", bufs=1) as constp,
            tc.tile_pool(name="wostream", bufs=1) as wop,
            tc.tile_pool(name="xstream", bufs=2) as xp,
            tc.tile_pool(name="qkv", bufs=NP) as qkvp,
            tc.tile_pool(name="vaug", bufs=NJ) as vaugp,
            tc.tile_pool(name="ptile", bufs=4) as pp,
            tc.tile_pool(name="post", bufs=2) as postp,
            tc.tile_pool(name="cxn", bufs=2) as cxnp,
            tc.tile_pool(name="outsb", bufs=2) as outp,
            tc.tile_pool(name="ps_s", bufs=2, space="PSUM") as ps_s,
            tc.tile_pool(name="ps_ctx", bufs=1, space="PSUM") as ps_ctx,
            tc.tile_pool(name="ps_misc", bufs=2, space="PSUM") as ps_misc,
            tc.tile_pool(name="dram", bufs=1, space="DRAM") as dramp,
        ):
            # ---- constants; x pair 0 split per-chunk unblocks the PE early.
            # x rides the Scalar ring, consts the Sync ring (parallel rings).
            # x pair 0 rides the Act ring (idle at startup) so its issues run
            # in parallel with the consts on the Sync ring
            xt0 = xp.tile([128, 8, PT], bf16, tag="xt")
            for o in range(8):      # 8 x 256KB: finer grains land earlier
                nc.scalar.dma_start(xt0[:, o, :], xTp_d[0][0][:, o, :])
            wT = constp.tile([128, 8, 3, 128], bf16, tag="wT")
            nc.sync.dma_start(wT[:], wT_d)
            ident = constp.tile([128, 128], bf16, tag="ident")
            nc.sync.dma_start(ident[:], ident_d)
            bqkv = constp.tile([128, 3], f32, tag="bqkv")
            nc.sync.dma_start(bqkv[:], bqkv_d)
            trimask = constp.tile([128, 128], bf16, tag="trimask")
            nc.sync.dma_start(trimask[:], trimask_d)

            # W_o / b_o ride the gpsimd ring once, before any collectives
            wo_sb = wop.tile([128, 8, 1024], bf16, tag="wo")
            nc.gpsimd.dma_start(wo_sb[:], woT_d)
            bo_row = wop.tile([1, 1024], f32, tag="bor")
            nc.gpsimd.dma_start(bo_row[:], bo_d)
            bo_sb = wop.tile([128, 1024], f32, tag="bobc")
            nc.gpsimd.partition_broadcast(bo_sb[:], bo_row[:], channels=128)

            # per-pair Q/K/V (transposed, [128, 1024]) and per-tile augmented V
            qkv_t = [[None] * NP for _ in range(3)]
            vaug_t = [[None] * NJ for _ in range(B)]

            # four a2a groups; dst core c <- its 128-token slice of each group.
            # DISTINCT tags: same-tag pool tiles alias one ring slot, which
            # would serialize ships of group k+1 behind the collective read
            # of group k.
            a2a_in = [dramp.tile([NCORE, 128, 128], bf16, name=f"a2a_in{k}",
                                 tag=f"a2a_in{k}")
                      for k in range(4)]
            a2a_out = [dramp.tile([NCORE, 128, 128], bf16, name=f"a2a_out{k}",
                                  tag=f"a2a_out{k}")
                       for k in range(4)]

            def proj_pair(p):
                if p == 0:
                    xt = xt0
                else:
                    xt = xp.tile([128, 8, PT], bf16, tag="xt")
                    for g in range(2):   # 2 x 1MB halves, separately gated
                        nc.sync.dma_start(xt[:, 4 * g:4 * (g + 1), :],
                                          xTp_d[p][g][:])
                for j in range(3):
                    qt = qkvp.tile([128, PT], bf16, tag=f"qkv{j}",
                                   name=f"qkv{j}_{p}")
                    # one matmul may write at most one 2KB PSUM bank (512
                    # fp32), so each 1024-token pair projects in two halves
                    for half in range(2):
                        ps = ps_misc.tile([128, TT], f32, tag="mm")
                        for o in range(8):
                            nc.tensor.matmul(
                                ps[:], wT[:, o, j, :],
                                xt[:, o, half * TT:(half + 1) * TT],
                                start=(o == 0), stop=(o == 7))
                        nc.vector.tensor_scalar_add(
                            qt[:, half * TT:(half + 1) * TT], ps[:],
                            bqkv[:, j:j + 1])
                    qkv_t[j][p] = qt

            def qslice(j, t, lo, hi):
                return qkv_t[j][t // 2][:, (t % 2) * TT + lo:(t % 2) * TT + hi]

            def vtrans_tile(t):
                b, tl = t // NJ, t % NJ
                # both heads in one tile, each head's ones-column at the END
                # of its 65-wide block, so one strided copy fills both heads
                va = vaugp.tile([128, NJ, 2 * (DK + 1)], bf16, tag=f"va{b}",
                                name=f"va{b}_{tl}")
                nc.vector.memset(
                    va[:].rearrange("p k (g c) -> p k g c", g=2)[:, :, :, DK:DK + 1],
                    1.0)
                vaug_t[b][tl] = va
                for kt in range(NJ):
                    ps_t = ps_misc.tile([128, TT], bf16, tag="mm")
                    nc.tensor.transpose(ps_t[:, 0:128],
                                        qslice(2, t, kt * KC, (kt + 1) * KC),
                                        ident[:])
                    nc.vector.tensor_copy(
                        va[:, kt, :].rearrange("p (g c) -> p g c", g=2)[:, :, 0:DK],
                        ps_t[:, 0:128].rearrange("p (g c) -> p g c", g=2))

            def attention_qtile(b, j, mid_hook=None, last=False):
                nk = 4 * (j + 1)
                pc = [ps_ctx.tile([DK + 1, TT], f32, tag=f"c{h}", name=f"pc{h}")
                      for h in range(2)]

                def emit_pv(p_tile, m):
                    # the masked query range of a diagonal chunk is all-zero
                    # P - skip streaming it (m == 0 is always full range)
                    q0 = max(m - 4 * j, 0) * KC
                    for h in range(2):
                        nc.tensor.matmul(
                            pc[h][:, q0:],
                            vaug_t[b][m // 4][:, m % 4,
                                              (DK + 1) * h:(DK + 1) * (h + 1)],
                            p_tile[:, TT * h + q0:TT * (h + 1)],
                            start=(m == 0), stop=(m == nk - 1),
                            skip_group_check=True)

                pending = []
                for m in range(nk):
                    tk = b * NJ + m // 4
                    ko = (m % 4) * KC
                    # queries below the diagonal chunk's start are masked out
                    # anyway - don't stream them through the PE
                    q0 = max(m - 4 * j, 0) * KC
                    ps = ps_s.tile([128, 2 * TT], f32, tag="s")
                    nc.tensor.matmul(ps[:, q0:TT],
                                     qslice(1, tk, ko, ko + KC)[0:DK, :],
                                     qslice(0, b * NJ + j, q0, TT)[0:DK, :],
                                     start=True, stop=True, tile_position=(0, 0))
                    nc.tensor.matmul(ps[:, TT + q0:],
                                     qslice(1, tk, ko, ko + KC)[DK:128, :],
                                     qslice(0, b * NJ + j, q0, TT)[DK:128, :],
                                     start=True, stop=True, tile_position=(64, 0))
                    p = pp.tile([128, 2 * TT], bf16, tag="p")
                    r = m - 4 * j
                    if r >= 0:
                        # cols [0, KC*r) are never streamed by emit_pv (its
                        # q0 skips them), so they need no zeroing
                        nc.scalar.activation(
                            p[:].rearrange("k (h q) -> k h q", h=2)[:, :, KC * r:],
                            ps[:].rearrange("k (h q) -> k h q", h=2)[:, :, KC * r:],
                            EXP, scale=float(SCALE))
                        nc.vector.tensor_tensor(
                            p[:].rearrange("k (h q) -> k h q", h=2)[:, :, KC * r:KC * (r + 1)],
                            p[:].rearrange("k (h q) -> k h q", h=2)[:, :, KC * r:KC * (r + 1)],
                            trimask[:, None, :].to_broadcast([128, 2, 128]), MULT)
                    else:
                        nc.scalar.activation(p[:], ps[:], EXP, scale=float(SCALE))
                    pending.append((p, m))
                    if len(pending) > 2:   # depth-2: PE never waits on a fresh exp
                        emit_pv(*pending.pop(0))
                    if m == 3 and mid_hook is not None:
                        mid_hook()   # e.g. late norm finish + a2a trigger
                for pm in pending:
                    emit_pv(*pm)

                if last:
                    # the final q-tile's norm isn't deferred and nothing
                    # recycles its PSUM banks: finish_norm_last reads pc
                    # directly (no cx copy, no DVE reciprocal chain)
                    return {"pc": pc, "b": b, "j": j}

                # normalization phase 1 (phase 2 deferred via finish_norm):
                # the rowsum row is spread across 32 DVE lanes via a block-
                # transpose, reciprocal'd batched ([32, 16] view instead of a
                # 1-lane [1, 512] at ~3.3us), and transposed back - all on
                # the DVE, so no DMA ever races collective channel traffic.
                cxs, rrows = [], []
                for h in range(2):
                    rt = cxnp.tile([32, TT], f32, tag="rt")
                    nc.vector.tensor_copy(rt[0:1, :], pc[h][DK:DK + 1, :])
                    # cx lives until finish_norm two iterations later, so two
                    # q-tiles' worth of cx tiles (2 heads each) coexist
                    cx = cxnp.tile([DK, TT], f32, tag="cx", bufs=4)
                    nc.vector.tensor_copy(cx[:], pc[h][0:DK, :])
                    cxs.append(cx)
                    rtT = cxnp.tile([32, TT], f32, tag="rtT")
                    nc.vector.transpose(rtT[:], rt[:])
                    rcT = cxnp.tile([32, TT], bf16, tag="rcT")
                    with nc.allow_low_precision(reason="softmax denominator"):
                        nc.vector.reciprocal(
                            rcT[:].rearrange("p (b c) -> p b c", c=32)[:, :, 0:1],
                            rtT[:].rearrange("p (b c) -> p b c", c=32)[:, :, 0:1])
                    rrow = cxnp.tile([32, TT], bf16, tag="rrow", bufs=4)
                    nc.vector.transpose(rrow[:], rcT[:])
                    rrows.append(rrow)   # row 0 = per-query reciprocal
                return {"cxs": cxs, "rrows": rrows, "b": b, "j": j}

            def finish_norm(st):
                # phase 2: Pool broadcasts each head's reciprocal row across
                # its 64 partitions (keeps the PE out of the norm chain),
                # then one multiply per head and one 3D-pattern ship per head
                # (dma_start issue is the scarce resource: ~0.63us each
                # through the shared HWDGE).
                b, j = st["b"], st["j"]
                k = A_OF[(b, j)]
                bcs = []
                for h in range(2):
                    bc = cxnp.tile([DK, TT], bf16, tag=f"bc{h}")
                    nc.gpsimd.partition_broadcast(
                        bc[:], st["rrows"][h][0:1, :], channels=DK)
                    bcs.append(bc)
                g0 = G0_OF[(b, j)]
                for h in range(2):
                    cxn = cxnp.tile([DK, TT], bf16, tag="cxn")
                    nc.vector.tensor_tensor(cxn[:], st["cxs"][h][:],
                                            bcs[h][:], MULT)
                    nc.sync.dma_start(
                        a2a_in[k][g0:g0 + 4, DK * h:DK * (h + 1), :]
                        .rearrange("g p c -> p g c"),
                        cxn[:].rearrange("p (g c) -> p g c", g=4))

            def finish_norm_last(st):
                # tail-latency path for the final q-tile: Act (idle after the
                # last exp) reciprocals the rowsum rows straight out of PSUM,
                # Pool broadcasts them, and the DVE multiply reads the
                # context numerator directly from PSUM - ships fire ~8us
                # sooner than the deferred-norm chain would manage.
                b, j = st["b"], st["j"]
                k, g0 = A_OF[(b, j)], G0_OF[(b, j)]
                pc = st["pc"]
                for h in range(2):
                    rt = cxnp.tile([32, TT], f32, tag="rt")
                    nc.vector.tensor_copy(rt[0:1, :], pc[h][DK:DK + 1, :])
                    rtT = cxnp.tile([32, TT], f32, tag="rtT")
                    nc.vector.transpose(rtT[:], rt[:])
                    rcT = cxnp.tile([32, TT], bf16, tag="rcT")
                    with nc.allow_low_precision(reason="softmax denominator"):
                        nc.vector.reciprocal(
                            rcT[:].rearrange("p (b c) -> p b c", c=32)[:, :, 0:1],
                            rtT[:].rearrange("p (b c) -> p b c", c=32)[:, :, 0:1])
                    rrow = cxnp.tile([32, TT], bf16, tag=f"rr{h}")
                    nc.vector.transpose(rrow[:], rcT[:])
                    bc = cxnp.tile([DK, TT], bf16, tag=f"bc{h}")
                    nc.gpsimd.partition_broadcast(bc[:], rrow[0:1, :],
                                                  channels=DK)
                    cxn = cxnp.tile([DK, TT], bf16, tag="cxn")
                    nc.vector.tensor_tensor(cxn[:], pc[h][0:DK, :], bc[:], MULT)
                    nc.sync.dma_start(
                        a2a_in[k][g0:g0 + 4, DK * h:DK * (h + 1), :]
                        .rearrange("g p c -> p g c"),
                        cxn[:].rearrange("p (g c) -> p g c", g=4))

            def do_a2a(k):
                nc.gpsimd.collective_compute(
                    "AllToAll", mybir.AluOpType.bypass,
                    replica_groups=[list(range(NCORE))],
                    ins=[a2a_in[k][:].opt()], outs=[a2a_out[k][:].opt()])

            ctx_tiles = {}

            def load_ctx(k, eng):
                # ctx(0)/(1) prefetch mid-kernel on the Sync ring (their
                # collectives are long done, and transfers avoid the tail's
                # collective channel traffic); ctx(2)/(3) load at the tail
                # on the Scalar ring (exps finished, Sync stays clear for
                # ships(7) -> a2a(3))
                ctx_sb = constp.tile([128, 8, 128], bf16, tag=f"ctx{k}",
                                     name=f"ctx{k}")
                eng.dma_start(ctx_sb[:],
                              a2a_out[k][:].rearrange("d p c -> p d c"))
                ctx_tiles[k] = ctx_sb

            def outproj_quarter(k, store_eng, split_store=False):
                ctx_sb = ctx_tiles[k]
                ot = outp.tile([128, 1024], bf16, tag="ot")
                for oh in range(2):      # 512-wide od halves (PSUM bank limit)
                    ps = ps_misc.tile([128, TT], f32, tag="mm")
                    for d in range(8):
                        nc.tensor.matmul(
                            ps[:], ctx_sb[:, d, :],
                            wo_sb[:, d, TT * oh:TT * (oh + 1)],
                            start=(d == 0), stop=(d == 7))
                    nc.vector.tensor_tensor(
                        ot[:, TT * oh:TT * (oh + 1)], ps[:],
                        bo_sb[:, TT * oh:TT * (oh + 1)],
                        mybir.AluOpType.add)
                    if split_store:  # ship each half as soon as its add lands
                        store_eng.dma_start(
                            outT_d[k, :, TT * oh:TT * (oh + 1)],
                            ot[:, TT * oh:TT * (oh + 1)])
                if not split_store:
                    store_eng.dma_start(outT_d[k], ot[:])

            # ---- pipelined schedule. Projection pairs run ahead of their
            # consumers; norm(q) finishes at iteration q+2 (a full iteration
            # of slack, so its PE outer-product never waits on the reciprocal
            # chain); collective triggers (gpsimd ring) fire as soon as both
            # contributing ships are in.
            proj_pair(0)                     # tiles 0,1 (b0 j0/j1)
            norms = {}
            for i in range(NT):
                if i == 0:
                    proj_pair(2)             # tiles 4,5 (b1 j0/j1)
                elif i == 2:
                    proj_pair(1)             # tiles 2,3 (b0 j2/j3)
                elif i == 4:
                    # pair 3 is first consumed at i=5 (vtrans tile 6); the
                    # late slot gives its restage an extra iteration of slack
                    proj_pair(3)             # tiles 6,7 (b1 j2/j3)
                if i >= 2:
                    finish_norm(norms.pop(i - 2))
                if i == 6:
                    load_ctx(0, nc.sync)     # a2a(0) done an iteration ago
                elif i == 7:
                    # G2 = both j=2 tiles; norm(5) just finished (i>=2 rule),
                    # so a2a(2) flies DURING the last (longest) attention
                    # tile and ctx(2) lands mid-i7
                    do_a2a(2)                # ships of q-tiles 4 (i6) + 5 (i7)
                    finish_norm(norms.pop(6))
                    load_ctx(1, nc.sync)     # a2a(1) done an iteration ago
                    load_ctx(2, nc.sync)
                if i == 4:
                    do_a2a(0)                # ships of q-tiles 0 (i2) + 2 (i4)
                elif i == 5:
                    do_a2a(1)                # ships of q-tiles 1 (i3) + 3 (i5)
                vtrans_tile(TILE_OF[i])
                norms[i] = attention_qtile(*ORDER[i], last=(i == NT - 1))
            # quarters 0-1 are emitted BEFORE the last norm (no semaphore-
            # counter dep on its Pool broadcasts -> they fill the PE right
            # after the last PV, including on the straggler core); their
            # stores ride the Act ring so ships(7) aren't queued behind
            # them on Sync. Quarter 2 (emitted after the trigger) fills the
            # PE while a2a(3) flies.
            outproj_quarter(0, nc.scalar)
            outproj_quarter(1, nc.scalar)
            finish_norm_last(norms.pop(7))
            do_a2a(3)                        # ships of q-tiles 6 (i6) + 7
            outproj_quarter(2, nc.sync)
            load_ctx(3, nc.scalar)
            outproj_quarter(3, nc.scalar, split_store=True)

    nc.compile()
    return nc


def make_in_maps(x, Wq, bq, Wk, bk, Wv, bv, Wo, bo):
    x = np.asarray(x, np.float32)
    xT = np.ascontiguousarray(x.reshape(T, D).T)                  # [D, T]
    # [NP, 128, 8, PT]: xTp[p, part, o, q] = xT[o*128+part, p*PT+q]
    xTp = np.ascontiguousarray(
        xT.reshape(8, 128, NP, PT).transpose(2, 1, 0, 3)).astype(npbf)

    woT = np.ascontiguousarray(
        np.asarray(Wo, np.float32).T.reshape(8, 128, 1024)
        .transpose(1, 0, 2)).astype(npbf)
    bo_row = np.ascontiguousarray(np.asarray(bo, np.float32)[None, :])

    trimask = (np.arange(128)[:, None] <= np.arange(128)[None, :]).astype(npbf)
    ident = np.eye(128, dtype=npbf)

    in_maps = []
    for c in range(NCORE):
        sl = slice(128 * c, 128 * (c + 1))
        wT_c = np.stack(
            [np.ascontiguousarray(
                np.asarray(W, np.float32)[sl, :].T.reshape(8, 128, 128)
                .transpose(1, 0, 2))
             for W in (Wq, Wk, Wv)], axis=2)                       # [128, 8, 3, 128]
        bqkv_c = np.stack([np.asarray(b_, np.float32)[sl]
                           for b_ in (bq, bk, bv)], axis=1)        # [128, 3]
        # dict order ~ restage order, by first consumption (see the
        # dram_tensor declarations)
        in_maps.append({
            "xTp0": np.ascontiguousarray(xTp[0]),
            "wT": np.ascontiguousarray(wT_c).astype(npbf),
            "bqkv": np.ascontiguousarray(bqkv_c),
            "xTp2a": np.ascontiguousarray(xTp[2, :, 0:4]),
            "xTp2b": np.ascontiguousarray(xTp[2, :, 4:8]),
            "ident": ident,
            "trimask": trimask,
            "xTp1a": np.ascontiguousarray(xTp[1, :, 0:4]),
            "xTp1b": np.ascontiguousarray(xTp[1, :, 4:8]),
            "xTp3a": np.ascontiguousarray(xTp[3, :, 0:4]),
            "xTp3b": np.ascontiguousarray(xTp[3, :, 4:8]),
            "bo": bo_row,
            "woT": woT,
        })
    return in_maps


def group_token(k, c):
    """(batch, seq start) of core c's 128-token slice of a2a group k."""
    if k < 2:
        return k, 128 * c
    return c // 4, 512 * k + 128 * (c % 4)


def assemble_output(results):
    # results[c]["outT"]: [4, 128, 1024]; slice k covers group_token(k, c)
    out = np.empty((B, S, D), np.float32)
    for c in range(NCORE):
        for k in range(4):
            b, t0 = group_token(k, c)
            out[b, t0:t0 + 128, :] = np.asarray(results[c]["outT"][k], np.float32)
    return out


_PROGRAM = None


def get_program():
    global _PROGRAM
    if _PROGRAM is None:
        _PROGRAM = build_program()
    return _PROGRAM


def run(in_maps, **kwargs):
    nc = get_program()
    return run_bass_kernel_spmd(nc, in_maps, core_ids=list(range(NCORE)), **kwargs)


def kernel(x, Wq, bq, Wk, bk, Wv, bv, Wo, bo):
    in_maps = make_in_maps(x, Wq, bq, Wk, bk, Wv, bv, Wo, bo)
    res = run(in_maps)
    return assemble_output(res.results)


if __name__ == "__main__":
    rng = np.random.default_rng(0)
    x = rng.standard_normal((B, S, D), dtype=np.float32)
    mk = lambda *s: ((rng.random(s).astype(np.float32)) - 0.5) / 16
    out = kernel(x, mk(D, D), mk(D), mk(D, D), mk(D), mk(D, D), mk(D),
                 mk(D, D), mk(D))
    print(out.shape, out.dtype, np.abs(out).mean())



# revision 44
# speedup vs baseline: 1.0687x; 1.0687x over previous
"""Multi-head causal attention (B=2, S=2048, D=1024, H=16) on 8 trn2 NeuronCores.

Strategy (tensor-parallel over heads, per the sharding hint):
  - Each core owns 2 heads (128 of 1024 hidden dims): W_q/W_k/W_v column-parallel.
  - Activations kept transposed ([dim, token]) end to end so every matmul
    contracts on the partition axis with zero on-device transposes of x.
  - Projections run per 1024-token pair of tiles (x loaded in 2MB chunks);
    each matmul streams 512 tokens (one fp32 PSUM bank); attention q-tiles
    are 512 wide. Diagonal-chunk score/PV matmuls stream only the unmasked
    query range.
  - scores^T = K^T.T @ Q^T per 128-key-chunk x 512-query-tile, two heads packed
    into disjoint PE row-groups (contraction is only dk=64).
  - softmax without max-subtraction (scores are O(1)); rowsum folded into the
    PV matmul via an augmented V [keys, 64+1] whose last column is ones.
  - exp only on the causal part of diagonal chunks; the rest of the P tile is
    zeroed, and only the 128-wide diagonal strip is tri-masked.
  - normalization: rowsum rows gathered to [128, 8] for one 128-lane DVE
    reciprocal, scattered back, broadcast via a PE outer-product; the finish
    (broadcast+multiply+ship) is deferred >= one full iteration so the PE
    never waits on the chain.
  - q-tiles processed batch-interleaved (b0j0, b1j0, b0j1, ...) and ctx
    re-sharded token-parallel with FOUR AllToAlls (one per half-batch); the
    gpsimd ring carries ONLY the collective triggers, so a busy CC engine
    can never stall compute; each a2a DRAM buffer has its own pool tag
    (shared-tag tiles alias one slot and serialize ships behind collectives).
  - out-projection (full W_o) per 128-token quarter at the tail, filling
    the PE while a2a(2)/(3) fly; only the last a2a + one quarter is exposed.
  - bf16 matmul inputs everywhere; PSUM accumulation stays fp32; the
    softmax reciprocal and the final output are bf16.

kernel(**inputs) takes the full unsharded inputs and returns the full output.
"""

import numpy as np
import ml_dtypes

import concourse.bass as bass
import concourse.mybir as mybir
import concourse.tile as tile
from concourse import bacc
from concourse.bass_utils import run_bass_kernel_spmd

B, S, D = 2, 2048, 1024
H, DK = 16, 64
NCORE = 8
T = B * S          # 4096 tokens
TT = 512           # attention q-tile width
PT = 1024          # projection pair width
NT = T // TT       # 8 token tiles
NP = T // PT       # 4 projection pairs
KC = 128           # key chunk
NJ = S // TT       # 4 q-tiles per batch
SCALE = 1.0 / np.sqrt(DK)

# batch-interleaved q-tile order; ORDER[i] = (b, j), its token tile is b*NJ+j
ORDER = [(0, 0), (1, 0), (0, 1), (1, 1), (0, 2), (1, 2), (0, 3), (1, 3)]
TILE_OF = [b * NJ + j for (b, j) in ORDER]
# a2a group of q-tile (b, j); groups pair tiles that finish adjacently so
# each a2a's inputs complete as early as possible: G0/G1 = j<2 per batch,
# G2 = both j=2 tiles (done by i5), G3 = both j=3 tiles (the tail pair)
A_OF = {(b, j): (b if j < 2 else j) for (b, j) in ORDER}
# dst slab base within a group: j<2 -> by j, j>=2 -> by batch
G0_OF = {(b, j): (4 * j if j < 2 else 4 * b) for (b, j) in ORDER}

f32 = mybir.dt.float32
bf16 = mybir.dt.bfloat16
EXP = mybir.ActivationFunctionType.Exp
MULT = mybir.AluOpType.mult
npbf = ml_dtypes.bfloat16


def build_program():
    nc = bacc.Bacc("TRN2", target_bir_lowering=False, debug=False,
                   num_devices=NCORE)

    # declaration order ~ host restage order: xTp0 + wT first (they gate the
    # first projection), xTp1-3 stream behind, woT last (tail-only)
    # restage follows declaration order; order by first consumption:
    # xTp0 + wT + bqkv gate the first projection, xTp2 is consumed at i=0,
    # ident/trimask at the first vtrans/diag chunk, xTp1 at i=2, xTp3 at
    # i=3, woT only at the tail
    def xin(name, shape):
        return nc.dram_tensor(name, shape, bf16, kind="ExternalInput").ap()

    xTp_d = [None] * NP
    xTp_d[0] = xin("xTp0", [128, 8, PT])
    wT_d = xin("wT", [128, 8, 3, 128])
    bqkv_d = nc.dram_tensor("bqkv", [128, 3], f32, kind="ExternalInput").ap()
    xTp_d[2] = xin("xTp2", [128, 8, PT])
    ident_d = xin("ident", [128, 128])
    trimask_d = xin("trimask", [128, 128])
    xTp_d[1] = xin("xTp1", [128, 8, PT])
    xTp_d[3] = xin("xTp3", [128, 8, PT])
    bo_d = nc.dram_tensor("bo", [1, 1024], f32, kind="ExternalInput").ap()
    woT_d = xin("woT", [128, 8, 1024])
    # outT[k] = this core's 128-token slice of a2a group k (see GROUP_TOK)
    outT_d = nc.dram_tensor("outT", [4, 128, 1024], bf16, kind="ExternalOutput").ap()

    with tile.TileContext(nc) as tc:
        with (
            tc.tile_pool(name="const", bufs=1) as constp,
            tc.tile_pool(name="wostream", bufs=1) as wop,
            tc.tile_pool(name="xstream", bufs=2) as xp,
            tc.tile_pool(name="qkv", bufs=NP) as qkvp,
            tc.tile_pool(name="vaug", bufs=NJ) as vaugp,
            tc.tile_pool(name="ptile", bufs=4) as pp,
            tc.tile_pool(name="post", bufs=2) as postp,
            tc.tile_pool(name="cxn", bufs=2) as cxnp,
            tc.tile_pool(name="outsb", bufs=2) as outp,
            tc.tile_pool(name="ps_s", bufs=2, space="PSUM") as ps_s,
            tc.tile_pool(name="ps_ctx", bufs=1, space="PSUM") as ps_ctx,
            tc.tile_pool(name="ps_misc", bufs=2, space="PSUM") as ps_misc,
            tc.tile_pool(name="dram", bufs=1, space="DRAM") as dramp,
        ):
            # ---- constants; x pair 0 split per-chunk unblocks the PE early.
            # x rides the Scalar ring, consts the Sync ring (parallel rings).
            # x pair 0 rides the Act ring (idle at startup) so its issues run
            # in parallel with the consts on the Sync ring
            xt0 = xp.tile([128, 8, PT], bf16, tag="xt")
            for o in range(8):      # 8 x 256KB: finer grains land earlier
                nc.scalar.dma_start(xt0[:, o, :], xTp_d[0][:, o, :])
            wT = constp.tile([128, 8, 3, 128], bf16, tag="wT")
            nc.sync.dma_start(wT[:], wT_d)
            ident = constp.tile([128, 128], bf16, tag="ident")
            nc.sync.dma_start(ident[:], ident_d)
            bqkv = constp.tile([128, 3], f32, tag="bqkv")
            nc.sync.dma_start(bqkv[:], bqkv_d)
            trimask = constp.tile([128, 128], bf16, tag="trimask")
            nc.sync.dma_start(trimask[:], trimask_d)

            # W_o / b_o ride the gpsimd ring once, before any collectives
            wo_sb = wop.tile([128, 8, 1024], bf16, tag="wo")
            nc.gpsimd.dma_start(wo_sb[:], woT_d)
            bo_row = wop.tile([1, 1024], f32, tag="bor")
            nc.gpsimd.dma_start(bo_row[:], bo_d)
            bo_sb = wop.tile([128, 1024], f32, tag="bobc")
            nc.gpsimd.partition_broadcast(bo_sb[:], bo_row[:], channels=128)

            # per-pair Q/K/V (transposed, [128, 1024]) and per-tile augmented V
            qkv_t = [[None] * NP for _ in range(3)]
            vaug_t = [[None] * NJ for _ in range(B)]

            # four a2a groups; dst core c <- its 128-token slice of each group.
            # DISTINCT tags: same-tag pool tiles alias one ring slot, which
            # would serialize ships of group k+1 behind the collective read
            # of group k.
            a2a_in = [dramp.tile([NCORE, 128, 128], bf16, name=f"a2a_in{k}",
                                 tag=f"a2a_in{k}")
                      for k in range(4)]
            a2a_out = [dramp.tile([NCORE, 128, 128], bf16, name=f"a2a_out{k}",
                                  tag=f"a2a_out{k}")
                       for k in range(4)]

            def proj_pair(p):
                if p == 0:
                    xt = xt0
                else:
                    xt = xp.tile([128, 8, PT], bf16, tag="xt")
                    for g in range(2):   # 2 x 1MB halves
                        nc.sync.dma_start(xt[:, 4 * g:4 * (g + 1), :],
                                          xTp_d[p][:, 4 * g:4 * (g + 1), :])
                for j in range(3):
                    qt = qkvp.tile([128, PT], bf16, tag=f"qkv{j}",
                                   name=f"qkv{j}_{p}")
                    # one matmul may write at most one 2KB PSUM bank (512
                    # fp32), so each 1024-token pair projects in two halves
                    for half in range(2):
                        ps = ps_misc.tile([128, TT], f32, tag="mm")
                        for o in range(8):
                            nc.tensor.matmul(
                                ps[:], wT[:, o, j, :],
                                xt[:, o, half * TT:(half + 1) * TT],
                                start=(o == 0), stop=(o == 7))
                        nc.vector.tensor_scalar_add(
                            qt[:, half * TT:(half + 1) * TT], ps[:],
                            bqkv[:, j:j + 1])
                    qkv_t[j][p] = qt

            def qslice(j, t, lo, hi):
                return qkv_t[j][t // 2][:, (t % 2) * TT + lo:(t % 2) * TT + hi]

            def vtrans_tile(t):
                b, tl = t // NJ, t % NJ
                # both heads in one tile, each head's ones-column at the END
                # of its 65-wide block, so one strided copy fills both heads
                va = vaugp.tile([128, NJ, 2 * (DK + 1)], bf16, tag=f"va{b}",
                                name=f"va{b}_{tl}")
                nc.vector.memset(
                    va[:].rearrange("p k (g c) -> p k g c", g=2)[:, :, :, DK:DK + 1],
                    1.0)
                vaug_t[b][tl] = va
                for kt in range(NJ):
                    ps_t = ps_misc.tile([128, TT], bf16, tag="mm")
                    nc.tensor.transpose(ps_t[:, 0:128],
                                        qslice(2, t, kt * KC, (kt + 1) * KC),
                                        ident[:])
                    nc.vector.tensor_copy(
                        va[:, kt, :].rearrange("p (g c) -> p g c", g=2)[:, :, 0:DK],
                        ps_t[:, 0:128].rearrange("p (g c) -> p g c", g=2))

            def attention_qtile(b, j, mid_hook=None, last=False):
                nk = 4 * (j + 1)
                pc = [ps_ctx.tile([DK + 1, TT], f32, tag=f"c{h}", name=f"pc{h}")
                      for h in range(2)]

                def emit_pv(p_tile, m):
                    # the masked query range of a diagonal chunk is all-zero
                    # P - skip streaming it (m == 0 is always full range)
                    q0 = max(m - 4 * j, 0) * KC
                    for h in range(2):
                        nc.tensor.matmul(
                            pc[h][:, q0:],
                            vaug_t[b][m // 4][:, m % 4,
                                              (DK + 1) * h:(DK + 1) * (h + 1)],
                            p_tile[:, TT * h + q0:TT * (h + 1)],
                            start=(m == 0), stop=(m == nk - 1),
                            skip_group_check=True)

                pending = []
                for m in range(nk):
                    tk = b * NJ + m // 4
                    ko = (m % 4) * KC
                    # queries below the diagonal chunk's start are masked out
                    # anyway - don't stream them through the PE
                    q0 = max(m - 4 * j, 0) * KC
                    ps = ps_s.tile([128, 2 * TT], f32, tag="s")
                    nc.tensor.matmul(ps[:, q0:TT],
                                     qslice(1, tk, ko, ko + KC)[0:DK, :],
                                     qslice(0, b * NJ + j, q0, TT)[0:DK, :],
                                     start=True, stop=True, tile_position=(0, 0))
                    nc.tensor.matmul(ps[:, TT + q0:],
                                     qslice(1, tk, ko, ko + KC)[DK:128, :],
                                     qslice(0, b * NJ + j, q0, TT)[DK:128, :],
                                     start=True, stop=True, tile_position=(64, 0))
                    p = pp.tile([128, 2 * TT], bf16, tag="p")
                    r = m - 4 * j
                    if r >= 0:
                        # cols [0, KC*r) are never streamed by emit_pv (its
                        # q0 skips them), so they need no zeroing
                        nc.scalar.activation(
                            p[:].rearrange("k (h q) -> k h q", h=2)[:, :, KC * r:],
                            ps[:].rearrange("k (h q) -> k h q", h=2)[:, :, KC * r:],
                            EXP, scale=float(SCALE))
                        nc.vector.tensor_tensor(
                            p[:].rearrange("k (h q) -> k h q", h=2)[:, :, KC * r:KC * (r + 1)],
                            p[:].rearrange("k (h q) -> k h q", h=2)[:, :, KC * r:KC * (r + 1)],
                            trimask[:, None, :].to_broadcast([128, 2, 128]), MULT)
                    else:
                        nc.scalar.activation(p[:], ps[:], EXP, scale=float(SCALE))
                    pending.append((p, m))
                    if len(pending) > 2:   # depth-2: PE never waits on a fresh exp
                        emit_pv(*pending.pop(0))
                    if m == 3 and mid_hook is not None:
                        mid_hook()   # e.g. late norm finish + a2a trigger
                for pm in pending:
                    emit_pv(*pm)

                if last:
                    # the final q-tile's norm isn't deferred and nothing
                    # recycles its PSUM banks: finish_norm_last reads pc
                    # directly (no cx copy, no DVE reciprocal chain)
                    return {"pc": pc, "b": b, "j": j}

                # normalization phase 1 (phase 2 deferred via finish_norm):
                # the rowsum row is spread across 32 DVE lanes via a block-
                # transpose, reciprocal'd batched ([32, 16] view instead of a
                # 1-lane [1, 512] at ~3.3us), and transposed back - all on
                # the DVE, so no DMA ever races collective channel traffic.
                cxs, rrows = [], []
                for h in range(2):
                    rt = cxnp.tile([32, TT], f32, tag="rt")
                    nc.vector.tensor_copy(rt[0:1, :], pc[h][DK:DK + 1, :])
                    # cx lives until finish_norm two iterations later, so two
                    # q-tiles' worth of cx tiles (2 heads each) coexist
                    cx = cxnp.tile([DK, TT], f32, tag="cx", bufs=4)
                    nc.vector.tensor_copy(cx[:], pc[h][0:DK, :])
                    cxs.append(cx)
                    rtT = cxnp.tile([32, TT], f32, tag="rtT")
                    nc.vector.transpose(rtT[:], rt[:])
                    rcT = cxnp.tile([32, TT], bf16, tag="rcT")
                    with nc.allow_low_precision(reason="softmax denominator"):
                        nc.vector.reciprocal(
                            rcT[:].rearrange("p (b c) -> p b c", c=32)[:, :, 0:1],
                            rtT[:].rearrange("p (b c) -> p b c", c=32)[:, :, 0:1])
                    rrow = cxnp.tile([32, TT], bf16, tag="rrow", bufs=4)
                    nc.vector.transpose(rrow[:], rcT[:])
                    rrows.append(rrow)   # row 0 = per-query reciprocal
                return {"cxs": cxs, "rrows": rrows, "b": b, "j": j}

            def finish_norm(st):
                # phase 2: Pool broadcasts each head's reciprocal row across
                # its 64 partitions (keeps the PE out of the norm chain),
                # then one multiply per head and one 3D-pattern ship per head
                # (dma_start issue is the scarce resource: ~0.63us each
                # through the shared HWDGE).
                b, j = st["b"], st["j"]
                k = A_OF[(b, j)]
                bcs = []
                for h in range(2):
                    bc = cxnp.tile([DK, TT], bf16, tag=f"bc{h}")
                    nc.gpsimd.partition_broadcast(
                        bc[:], st["rrows"][h][0:1, :], channels=DK)
                    bcs.append(bc)
                g0 = G0_OF[(b, j)]
                for h in range(2):
                    cxn = cxnp.tile([DK, TT], bf16, tag="cxn")
                    nc.vector.tensor_tensor(cxn[:], st["cxs"][h][:],
                                            bcs[h][:], MULT)
                    nc.sync.dma_start(
                        a2a_in[k][g0:g0 + 4, DK * h:DK * (h + 1), :]
                        .rearrange("g p c -> p g c"),
                        cxn[:].rearrange("p (g c) -> p g c", g=4))

            def finish_norm_last(st):
                # tail-latency path for the final q-tile: Act (idle after the
                # last exp) reciprocals the rowsum rows straight out of PSUM,
                # Pool broadcasts them, and the DVE multiply reads the
                # context numerator directly from PSUM - ships fire ~8us
                # sooner than the deferred-norm chain would manage.
                b, j = st["b"], st["j"]
                k, g0 = A_OF[(b, j)], G0_OF[(b, j)]
                pc = st["pc"]
                for h in range(2):
                    rt = cxnp.tile([32, TT], f32, tag="rt")
                    nc.vector.tensor_copy(rt[0:1, :], pc[h][DK:DK + 1, :])
                    rtT = cxnp.tile([32, TT], f32, tag="rtT")
                    nc.vector.transpose(rtT[:], rt[:])
                    rcT = cxnp.tile([32, TT], bf16, tag="rcT")
                    with nc.allow_low_precision(reason="softmax denominator"):
                        nc.vector.reciprocal(
                            rcT[:].rearrange("p (b c) -> p b c", c=32)[:, :, 0:1],
                            rtT[:].rearrange("p (b c) -> p b c", c=32)[:, :, 0:1])
                    rrow = cxnp.tile([32, TT], bf16, tag=f"rr{h}")
                    nc.vector.transpose(rrow[:], rcT[:])
                    bc = cxnp.tile([DK, TT], bf16, tag=f"bc{h}")
                    nc.gpsimd.partition_broadcast(bc[:], rrow[0:1, :],
                                                  channels=DK)
                    cxn = cxnp.tile([DK, TT], bf16, tag="cxn")
                    nc.vector.tensor_tensor(cxn[:], pc[h][0:DK, :], bc[:], MULT)
                    nc.sync.dma_start(
                        a2a_in[k][g0:g0 + 4, DK * h:DK * (h + 1), :]
                        .rearrange("g p c -> p g c"),
                        cxn[:].rearrange("p (g c) -> p g c", g=4))

            def do_a2a(k):
                nc.gpsimd.collective_compute(
                    "AllToAll", mybir.AluOpType.bypass,
                    replica_groups=[list(range(NCORE))],
                    ins=[a2a_in[k][:].opt()], outs=[a2a_out[k][:].opt()])

            ctx_tiles = {}

            def load_ctx(k, eng):
                # ctx(0)/(1) prefetch mid-kernel on the Sync ring (their
                # collectives are long done, and transfers avoid the tail's
                # collective channel traffic); ctx(2)/(3) load at the tail
                # on the Scalar ring (exps finished, Sync stays clear for
                # ships(7) -> a2a(3))
                ctx_sb = constp.tile([128, 8, 128], bf16, tag=f"ctx{k}",
                                     name=f"ctx{k}")
                eng.dma_start(ctx_sb[:],
                              a2a_out[k][:].rearrange("d p c -> p d c"))
                ctx_tiles[k] = ctx_sb

            def outproj_quarter(k, store_eng, split_store=False):
                ctx_sb = ctx_tiles[k]
                ot = outp.tile([128, 1024], bf16, tag="ot")
                for oh in range(2):      # 512-wide od halves (PSUM bank limit)
                    ps = ps_misc.tile([128, TT], f32, tag="mm")
                    for d in range(8):
                        nc.tensor.matmul(
                            ps[:], ctx_sb[:, d, :],
                            wo_sb[:, d, TT * oh:TT * (oh + 1)],
                            start=(d == 0), stop=(d == 7))
                    nc.vector.tensor_tensor(
                        ot[:, TT * oh:TT * (oh + 1)], ps[:],
                        bo_sb[:, TT * oh:TT * (oh + 1)],
                        mybir.AluOpType.add)
                    if split_store:  # ship each half as soon as its add lands
                        store_eng.dma_start(
                            outT_d[k, :, TT * oh:TT * (oh + 1)],
                            ot[:, TT * oh:TT * (oh + 1)])
                if not split_store:
                    store_eng.dma_start(outT_d[k], ot[:])

            # ---- pipelined schedule. Projection pairs run ahead of their
            # consumers; norm(q) finishes at iteration q+2 (a full iteration
            # of slack, so its PE outer-product never waits on the reciprocal
            # chain); collective triggers (gpsimd ring) fire as soon as both
            # contributing ships are in.
            proj_pair(0)                     # tiles 0,1 (b0 j0/j1)
            norms = {}
            for i in range(NT):
                if i == 0:
                    proj_pair(2)             # tiles 4,5 (b1 j0/j1)
                elif i == 2:
                    proj_pair(1)             # tiles 2,3 (b0 j2/j3)
                elif i == 4:
                    # pair 3 is first consumed at i=5 (vtrans tile 6); the
                    # late slot gives its restage an extra iteration of slack
                    proj_pair(3)             # tiles 6,7 (b1 j2/j3)
                if i >= 2:
                    finish_norm(norms.pop(i - 2))
                if i == 6:
                    load_ctx(0, nc.sync)     # a2a(0) done an iteration ago
                elif i == 7:
                    # G2 = both j=2 tiles; norm(5) just finished (i>=2 rule),
                    # so a2a(2) flies DURING the last (longest) attention
                    # tile and ctx(2) lands mid-i7
                    do_a2a(2)                # ships of q-tiles 4 (i6) + 5 (i7)
                    finish_norm(norms.pop(6))
                    load_ctx(1, nc.sync)     # a2a(1) done an iteration ago
                    load_ctx(2, nc.sync)
                if i == 4:
                    do_a2a(0)                # ships of q-tiles 0 (i2) + 2 (i4)
                elif i == 5:
                    do_a2a(1)                # ships of q-tiles 1 (i3) + 3 (i5)
                vtrans_tile(TILE_OF[i])
                norms[i] = attention_qtile(*ORDER[i], last=(i == NT - 1))
            # quarters 0-1 are emitted BEFORE the last norm (no semaphore-
            # counter dep on its Pool broadcasts -> they fill the PE right
            # after the last PV, including on the straggler core); their
            # stores ride the Act ring so ships(7) aren't queued behind
            # them on Sync. Quarter 2 (emitted after the trigger) fills the
            # PE while a2a(3) flies.
            outproj_quarter(0, nc.scalar)
            outproj_quarter(1, nc.scalar)
            finish_norm_last(norms.pop(7))
            do_a2a(3)                        # ships of q-tiles 6 (i6) + 7
            outproj_quarter(2, nc.sync)
            load_ctx(3, nc.scalar)
            outproj_quarter(3, nc.scalar, split_store=True)

    nc.compile()
    return nc


def make_in_maps(x, Wq, bq, Wk, bk, Wv, bv, Wo, bo):
    x = np.asarray(x, np.float32)
    xT = np.ascontiguousarray(x.reshape(T, D).T)                  # [D, T]
    # [NP, 128, 8, PT]: xTp[p, part, o, q] = xT[o*128+part, p*PT+q]
    xTp = np.ascontiguousarray(
        xT.reshape(8, 128, NP, PT).transpose(2, 1, 0, 3)).astype(npbf)

    woT = np.ascontiguousarray(
        np.asarray(Wo, np.float32).T.reshape(8, 128, 1024)
        .transpose(1, 0, 2)).astype(npbf)
    bo_row = np.ascontiguousarray(np.asarray(bo, np.float32)[None, :])

    trimask = (np.arange(128)[:, None] <= np.arange(128)[None, :]).astype(npbf)
    ident = np.eye(128, dtype=npbf)

    in_maps = []
    for c in range(NCORE):
        sl = slice(128 * c, 128 * (c + 1))
        wT_c = np.stack(
            [np.ascontiguousarray(
                np.asarray(W, np.float32)[sl, :].T.reshape(8, 128, 128)
                .transpose(1, 0, 2))
             for W in (Wq, Wk, Wv)], axis=2)                       # [128, 8, 3, 128]
        bqkv_c = np.stack([np.asarray(b_, np.float32)[sl]
                           for b_ in (bq, bk, bv)], axis=1)        # [128, 3]
        # dict order ~ restage order, by first consumption (see the
        # dram_tensor declarations)
        in_maps.append({
            "xTp0": np.ascontiguousarray(xTp[0]),
            "wT": np.ascontiguousarray(wT_c).astype(npbf),
            "bqkv": np.ascontiguousarray(bqkv_c),
            "xTp2": np.ascontiguousarray(xTp[2]),
            "ident": ident,
            "trimask": trimask,
            "xTp1": np.ascontiguousarray(xTp[1]),
            "xTp3": np.ascontiguousarray(xTp[3]),
            "bo": bo_row,
            "woT": woT,
        })
    return in_maps


def group_token(k, c):
    """(batch, seq start) of core c's 128-token slice of a2a group k."""
    if k < 2:
        return k, 128 * c
    return c // 4, 512 * k + 128 * (c % 4)


def assemble_output(results):
    # results[c]["outT"]: [4, 128, 1024]; slice k covers group_token(k, c)
    out = np.empty((B, S, D), np.float32)
    for c in range(NCORE):
        for k in range(4):
            b, t0 = group_token(k, c)
            out[b, t0:t0 + 128, :] = np.asarray(results[c]["outT"][k], np.float32)
    return out


_PROGRAM = None


def get_program():
    global _PROGRAM
    if _PROGRAM is None:
        _PROGRAM = build_program()
    return _PROGRAM


def run(in_maps, **kwargs):
    nc = get_program()
    return run_bass_kernel_spmd(nc, in_maps, core_ids=list(range(NCORE)), **kwargs)


def kernel(x, Wq, bq, Wk, bk, Wv, bv, Wo, bo):
    in_maps = make_in_maps(x, Wq, bq, Wk, bk, Wv, bv, Wo, bo)
    res = run(in_maps)
    return assemble_output(res.results)


if __name__ == "__main__":
    rng = np.random.default_rng(0)
    x = rng.standard_normal((B, S, D), dtype=np.float32)
    mk = lambda *s: ((rng.random(s).astype(np.float32)) - 0.5) / 16
    out = kernel(x, mk(D, D), mk(D), mk(D, D), mk(D), mk(D, D), mk(D),
                 mk(D, D), mk(D))
    print(out.shape, out.dtype, np.abs(out).mean())



# revision 45
# speedup vs baseline: 1.0759x; 1.0067x over previous
"""Multi-head causal attention (B=2, S=2048, D=1024, H=16) on 8 trn2 NeuronCores.

Strategy (tensor-parallel over heads, per the sharding hint):
  - Each core owns 2 heads (128 of 1024 hidden dims): W_q/W_k/W_v column-parallel.
  - Activations kept transposed ([dim, token]) end to end so every matmul
    contracts on the partition axis with zero on-device transposes of x.
  - Projections run per 1024-token pair of tiles (x loaded in 2MB chunks);
    each matmul streams 512 tokens (one fp32 PSUM bank); attention q-tiles
    are 512 wide. Diagonal-chunk score/PV matmuls stream only the unmasked
    query range.
  - scores^T = K^T.T @ Q^T per 128-key-chunk x 512-query-tile, two heads packed
    into disjoint PE row-groups (contraction is only dk=64).
  - softmax without max-subtraction (scores are O(1)); rowsum folded into the
    PV matmul via an augmented V [keys, 64+1] whose last column is ones.
  - exp only on the causal part of diagonal chunks; the rest of the P tile is
    zeroed, and only the 128-wide diagonal strip is tri-masked.
  - normalization: rowsum rows gathered to [128, 8] for one 128-lane DVE
    reciprocal, scattered back, broadcast via a PE outer-product; the finish
    (broadcast+multiply+ship) is deferred >= one full iteration so the PE
    never waits on the chain.
  - q-tiles processed batch-interleaved (b0j0, b1j0, b0j1, ...) and ctx
    re-sharded token-parallel with FOUR AllToAlls (one per half-batch); the
    gpsimd ring carries ONLY the collective triggers, so a busy CC engine
    can never stall compute; each a2a DRAM buffer has its own pool tag
    (shared-tag tiles alias one slot and serialize ships behind collectives).
  - out-projection (full W_o) per 128-token quarter at the tail, filling
    the PE while a2a(2)/(3) fly; only the last a2a + one quarter is exposed.
  - bf16 matmul inputs everywhere; PSUM accumulation stays fp32; the
    softmax reciprocal and the final output are bf16.

kernel(**inputs) takes the full unsharded inputs and returns the full output.
"""

import numpy as np
import ml_dtypes

import concourse.bass as bass
import concourse.mybir as mybir
import concourse.tile as tile
from concourse import bacc
from concourse.bass_utils import run_bass_kernel_spmd

B, S, D = 2, 2048, 1024
H, DK = 16, 64
NCORE = 8
T = B * S          # 4096 tokens
TT = 512           # attention q-tile width
PT = 1024          # projection pair width
NT = T // TT       # 8 token tiles
NP = T // PT       # 4 projection pairs
KC = 128           # key chunk
NJ = S // TT       # 4 q-tiles per batch
SCALE = 1.0 / np.sqrt(DK)

# batch-interleaved q-tile order; ORDER[i] = (b, j), its token tile is b*NJ+j
ORDER = [(0, 0), (1, 0), (0, 1), (1, 1), (0, 2), (1, 2), (0, 3), (1, 3)]
TILE_OF = [b * NJ + j for (b, j) in ORDER]
# a2a group of q-tile (b, j); groups pair tiles that finish adjacently so
# each a2a's inputs complete as early as possible: G0/G1 = j<2 per batch,
# G2 = both j=2 tiles (done by i5), G3 = both j=3 tiles (the tail pair)
A_OF = {(b, j): (b if j < 2 else j) for (b, j) in ORDER}
# dst slab base within a group: j<2 -> by j, j>=2 -> by batch
G0_OF = {(b, j): (4 * j if j < 2 else 4 * b) for (b, j) in ORDER}

f32 = mybir.dt.float32
bf16 = mybir.dt.bfloat16
EXP = mybir.ActivationFunctionType.Exp
MULT = mybir.AluOpType.mult
npbf = ml_dtypes.bfloat16


def build_program():
    nc = bacc.Bacc("TRN2", target_bir_lowering=False, debug=False,
                   num_devices=NCORE)

    # declaration order ~ host restage order: xTp0 + wT first (they gate the
    # first projection), xTp1-3 stream behind, woT last (tail-only)
    # restage follows declaration order; order by first consumption:
    # xTp0 + wT + bqkv gate the first projection, xTp2 is consumed at i=0,
    # ident/trimask at the first vtrans/diag chunk, xTp1 at i=2, xTp3 at
    # i=3, woT only at the tail
    def xin(name, shape):
        return nc.dram_tensor(name, shape, bf16, kind="ExternalInput").ap()

    xTp_d = [None] * NP
    xTp_d[0] = xin("xTp0", [128, 8, PT])
    wT_d = xin("wT", [128, 8, 3, 128])
    bqkv_d = nc.dram_tensor("bqkv", [128, 3], f32, kind="ExternalInput").ap()
    xTp_d[2] = xin("xTp2", [128, 8, PT])
    ident_d = xin("ident", [128, 128])
    trimask_d = xin("trimask", [128, 128])
    xTp_d[1] = xin("xTp1", [128, 8, PT])
    xTp_d[3] = xin("xTp3", [128, 8, PT])
    bo_d = nc.dram_tensor("bo", [1, 1024], f32, kind="ExternalInput").ap()
    woT_d = xin("woT", [128, 8, 1024])
    # outT[k] = this core's 128-token slice of a2a group k (see GROUP_TOK)
    outT_d = nc.dram_tensor("outT", [4, 128, 1024], bf16, kind="ExternalOutput").ap()

    with tile.TileContext(nc) as tc:
        with (
            tc.tile_pool(name="const", bufs=1) as constp,
            tc.tile_pool(name="wostream", bufs=1) as wop,
            tc.tile_pool(name="xstream", bufs=2) as xp,
            tc.tile_pool(name="qkv", bufs=NP) as qkvp,
            tc.tile_pool(name="vaug", bufs=NJ) as vaugp,
            tc.tile_pool(name="ptile", bufs=4) as pp,
            tc.tile_pool(name="post", bufs=2) as postp,
            tc.tile_pool(name="cxn", bufs=2) as cxnp,
            tc.tile_pool(name="outsb", bufs=2) as outp,
            tc.tile_pool(name="ps_s", bufs=2, space="PSUM") as ps_s,
            tc.tile_pool(name="ps_ctx", bufs=1, space="PSUM") as ps_ctx,
            tc.tile_pool(name="ps_misc", bufs=2, space="PSUM") as ps_misc,
            tc.tile_pool(name="dram", bufs=1, space="DRAM") as dramp,
        ):
            # ---- constants; x pair 0 split per-chunk unblocks the PE early.
            # x rides the Scalar ring, consts the Sync ring (parallel rings).
            # x pair 0 rides the Act ring (idle at startup) so its issues run
            # in parallel with the consts on the Sync ring
            xt0 = xp.tile([128, 8, PT], bf16, tag="xt")
            for o in range(8):      # 8 x 256KB: finer grains land earlier
                nc.scalar.dma_start(xt0[:, o, :], xTp_d[0][:, o, :])
            wT = constp.tile([128, 8, 3, 128], bf16, tag="wT")
            nc.sync.dma_start(wT[:], wT_d)
            ident = constp.tile([128, 128], bf16, tag="ident")
            nc.sync.dma_start(ident[:], ident_d)
            bqkv = constp.tile([128, 3], f32, tag="bqkv")
            nc.sync.dma_start(bqkv[:], bqkv_d)
            trimask = constp.tile([128, 128], bf16, tag="trimask")
            nc.sync.dma_start(trimask[:], trimask_d)

            # W_o / b_o ride the gpsimd ring once, before any collectives
            wo_sb = wop.tile([128, 8, 1024], bf16, tag="wo")
            nc.gpsimd.dma_start(wo_sb[:], woT_d)
            bo_row = wop.tile([1, 1024], f32, tag="bor")
            nc.gpsimd.dma_start(bo_row[:], bo_d)
            bo_sb = wop.tile([128, 1024], f32, tag="bobc")
            nc.gpsimd.partition_broadcast(bo_sb[:], bo_row[:], channels=128)

            # per-pair Q/K/V (transposed, [128, 1024]) and per-tile augmented V
            qkv_t = [[None] * NP for _ in range(3)]
            vaug_t = [[None] * NJ for _ in range(B)]

            # four a2a groups; dst core c <- its 128-token slice of each group.
            # DISTINCT tags: same-tag pool tiles alias one ring slot, which
            # would serialize ships of group k+1 behind the collective read
            # of group k.
            a2a_in = [dramp.tile([NCORE, 128, 128], bf16, name=f"a2a_in{k}",
                                 tag=f"a2a_in{k}")
                      for k in range(4)]
            a2a_out = [dramp.tile([NCORE, 128, 128], bf16, name=f"a2a_out{k}",
                                  tag=f"a2a_out{k}")
                       for k in range(4)]

            def proj_pair(p):
                if p == 0:
                    xt = xt0
                else:
                    xt = xp.tile([128, 8, PT], bf16, tag="xt")
                    for g in range(2):   # 2 x 1MB halves
                        nc.sync.dma_start(xt[:, 4 * g:4 * (g + 1), :],
                                          xTp_d[p][:, 4 * g:4 * (g + 1), :])
                for j in range(3):
                    qt = qkvp.tile([128, PT], bf16, tag=f"qkv{j}",
                                   name=f"qkv{j}_{p}")
                    # one matmul may write at most one 2KB PSUM bank (512
                    # fp32), so each 1024-token pair projects in two halves
                    for half in range(2):
                        ps = ps_misc.tile([128, TT], f32, tag="mm")
                        for o in range(8):
                            nc.tensor.matmul(
                                ps[:], wT[:, o, j, :],
                                xt[:, o, half * TT:(half + 1) * TT],
                                start=(o == 0), stop=(o == 7))
                        nc.vector.tensor_scalar_add(
                            qt[:, half * TT:(half + 1) * TT], ps[:],
                            bqkv[:, j:j + 1])
                    qkv_t[j][p] = qt

            def qslice(j, t, lo, hi):
                return qkv_t[j][t // 2][:, (t % 2) * TT + lo:(t % 2) * TT + hi]

            def vtrans_tile(t):
                b, tl = t // NJ, t % NJ
                # both heads in one tile, each head's ones-column at the END
                # of its 65-wide block, so one strided copy fills both heads
                va = vaugp.tile([128, NJ, 2 * (DK + 1)], bf16, tag=f"va{b}",
                                name=f"va{b}_{tl}")
                nc.vector.memset(
                    va[:].rearrange("p k (g c) -> p k g c", g=2)[:, :, :, DK:DK + 1],
                    1.0)
                vaug_t[b][tl] = va
                for kt in range(NJ):
                    ps_t = ps_misc.tile([128, TT], bf16, tag="mm")
                    nc.tensor.transpose(ps_t[:, 0:128],
                                        qslice(2, t, kt * KC, (kt + 1) * KC),
                                        ident[:])
                    nc.vector.tensor_copy(
                        va[:, kt, :].rearrange("p (g c) -> p g c", g=2)[:, :, 0:DK],
                        ps_t[:, 0:128].rearrange("p (g c) -> p g c", g=2))

            def attention_qtile(b, j, mid_hook=None, last=False):
                nk = 4 * (j + 1)
                pc = [ps_ctx.tile([DK + 1, TT], f32, tag=f"c{h}", name=f"pc{h}")
                      for h in range(2)]

                def emit_pv(p_tile, m):
                    # the masked query range of a diagonal chunk is all-zero
                    # P - skip streaming it (m == 0 is always full range)
                    q0 = max(m - 4 * j, 0) * KC
                    for h in range(2):
                        nc.tensor.matmul(
                            pc[h][:, q0:],
                            vaug_t[b][m // 4][:, m % 4,
                                              (DK + 1) * h:(DK + 1) * (h + 1)],
                            p_tile[:, TT * h + q0:TT * (h + 1)],
                            start=(m == 0), stop=(m == nk - 1),
                            skip_group_check=True)

                pending = []
                for m in range(nk):
                    tk = b * NJ + m // 4
                    ko = (m % 4) * KC
                    # queries below the diagonal chunk's start are masked out
                    # anyway - don't stream them through the PE
                    q0 = max(m - 4 * j, 0) * KC
                    ps = ps_s.tile([128, 2 * TT], f32, tag="s")
                    nc.tensor.matmul(ps[:, q0:TT],
                                     qslice(1, tk, ko, ko + KC)[0:DK, :],
                                     qslice(0, b * NJ + j, q0, TT)[0:DK, :],
                                     start=True, stop=True, tile_position=(0, 0))
                    nc.tensor.matmul(ps[:, TT + q0:],
                                     qslice(1, tk, ko, ko + KC)[DK:128, :],
                                     qslice(0, b * NJ + j, q0, TT)[DK:128, :],
                                     start=True, stop=True, tile_position=(64, 0))
                    p = pp.tile([128, 2 * TT], bf16, tag="p")
                    r = m - 4 * j
                    if r >= 0:
                        # cols [0, KC*r) are never streamed by emit_pv (its
                        # q0 skips them), so they need no zeroing
                        nc.scalar.activation(
                            p[:].rearrange("k (h q) -> k h q", h=2)[:, :, KC * r:],
                            ps[:].rearrange("k (h q) -> k h q", h=2)[:, :, KC * r:],
                            EXP, scale=float(SCALE))
                        nc.vector.tensor_tensor(
                            p[:].rearrange("k (h q) -> k h q", h=2)[:, :, KC * r:KC * (r + 1)],
                            p[:].rearrange("k (h q) -> k h q", h=2)[:, :, KC * r:KC * (r + 1)],
                            trimask[:, None, :].to_broadcast([128, 2, 128]), MULT)
                    else:
                        nc.scalar.activation(p[:], ps[:], EXP, scale=float(SCALE))
                    pending.append((p, m))
                    if len(pending) > 2:   # depth-2: PE never waits on a fresh exp
                        emit_pv(*pending.pop(0))
                    if m == 3 and mid_hook is not None:
                        mid_hook()   # e.g. late norm finish + a2a trigger
                for pm in pending:
                    emit_pv(*pm)

                if last:
                    # the final q-tile's norm isn't deferred and nothing
                    # recycles its PSUM banks: finish_norm_last reads pc
                    # directly (no cx copy, no DVE reciprocal chain)
                    return {"pc": pc, "b": b, "j": j}

                # normalization phase 1 (phase 2 deferred via finish_norm):
                # the rowsum row is spread across 32 DVE lanes via a block-
                # transpose, reciprocal'd batched ([32, 16] view instead of a
                # 1-lane [1, 512] at ~3.3us), and transposed back - all on
                # the DVE, so no DMA ever races collective channel traffic.
                cxs, rrows = [], []
                for h in range(2):
                    rt = cxnp.tile([32, TT], f32, tag="rt")
                    nc.vector.tensor_copy(rt[0:1, :], pc[h][DK:DK + 1, :])
                    # cx lives until finish_norm two iterations later, so two
                    # q-tiles' worth of cx tiles (2 heads each) coexist
                    cx = cxnp.tile([DK, TT], f32, tag="cx", bufs=4)
                    nc.vector.tensor_copy(cx[:], pc[h][0:DK, :])
                    cxs.append(cx)
                    rtT = cxnp.tile([32, TT], f32, tag="rtT")
                    nc.vector.transpose(rtT[:], rt[:])
                    rcT = cxnp.tile([32, TT], bf16, tag="rcT")
                    with nc.allow_low_precision(reason="softmax denominator"):
                        nc.vector.reciprocal(
                            rcT[:].rearrange("p (b c) -> p b c", c=32)[:, :, 0:1],
                            rtT[:].rearrange("p (b c) -> p b c", c=32)[:, :, 0:1])
                    rrow = cxnp.tile([32, TT], bf16, tag="rrow", bufs=4)
                    nc.vector.transpose(rrow[:], rcT[:])
                    rrows.append(rrow)   # row 0 = per-query reciprocal
                return {"cxs": cxs, "rrows": rrows, "b": b, "j": j}

            def finish_norm(st):
                # phase 2: Pool broadcasts each head's reciprocal row across
                # its 64 partitions (keeps the PE out of the norm chain),
                # then one multiply per head and one 3D-pattern ship per head
                # (dma_start issue is the scarce resource: ~0.63us each
                # through the shared HWDGE).
                b, j = st["b"], st["j"]
                k = A_OF[(b, j)]
                bcs = []
                for h in range(2):
                    bc = cxnp.tile([DK, TT], bf16, tag=f"bc{h}")
                    nc.gpsimd.partition_broadcast(
                        bc[:], st["rrows"][h][0:1, :], channels=DK)
                    bcs.append(bc)
                g0 = G0_OF[(b, j)]
                for h in range(2):
                    cxn = cxnp.tile([DK, TT], bf16, tag="cxn")
                    nc.vector.tensor_tensor(cxn[:], st["cxs"][h][:],
                                            bcs[h][:], MULT)
                    nc.sync.dma_start(
                        a2a_in[k][g0:g0 + 4, DK * h:DK * (h + 1), :]
                        .rearrange("g p c -> p g c"),
                        cxn[:].rearrange("p (g c) -> p g c", g=4))

            def finish_norm_last(st):
                # tail-latency path for the final q-tile: Act (idle after the
                # last exp) reciprocals the rowsum rows straight out of PSUM,
                # Pool broadcasts them, and the DVE multiply reads the
                # context numerator directly from PSUM - ships fire ~8us
                # sooner than the deferred-norm chain would manage.
                b, j = st["b"], st["j"]
                k, g0 = A_OF[(b, j)], G0_OF[(b, j)]
                pc = st["pc"]
                for h in range(2):
                    rt = cxnp.tile([32, TT], f32, tag="rt")
                    nc.vector.tensor_copy(rt[0:1, :], pc[h][DK:DK + 1, :])
                    rtT = cxnp.tile([32, TT], f32, tag="rtT")
                    nc.vector.transpose(rtT[:], rt[:])
                    rcT = cxnp.tile([32, TT], bf16, tag="rcT")
                    with nc.allow_low_precision(reason="softmax denominator"):
                        nc.vector.reciprocal(
                            rcT[:].rearrange("p (b c) -> p b c", c=32)[:, :, 0:1],
                            rtT[:].rearrange("p (b c) -> p b c", c=32)[:, :, 0:1])
                    rrow = cxnp.tile([32, TT], bf16, tag=f"rr{h}")
                    nc.vector.transpose(rrow[:], rcT[:])
                    bc = cxnp.tile([DK, TT], bf16, tag=f"bc{h}")
                    nc.gpsimd.partition_broadcast(bc[:], rrow[0:1, :],
                                                  channels=DK)
                    cxn = cxnp.tile([DK, TT], bf16, tag="cxn")
                    nc.vector.tensor_tensor(cxn[:], pc[h][0:DK, :], bc[:], MULT)
                    nc.sync.dma_start(
                        a2a_in[k][g0:g0 + 4, DK * h:DK * (h + 1), :]
                        .rearrange("g p c -> p g c"),
                        cxn[:].rearrange("p (g c) -> p g c", g=4))

            def do_a2a(k):
                nc.gpsimd.collective_compute(
                    "AllToAll", mybir.AluOpType.bypass,
                    replica_groups=[list(range(NCORE))],
                    ins=[a2a_in[k][:].opt()], outs=[a2a_out[k][:].opt()])

            ctx_tiles = {}

            def load_ctx(k, eng):
                # ctx(0)/(1) prefetch mid-kernel on the Sync ring (their
                # collectives are long done, and transfers avoid the tail's
                # collective channel traffic); ctx(2)/(3) load at the tail
                # on the Scalar ring (exps finished, Sync stays clear for
                # ships(7) -> a2a(3))
                ctx_sb = constp.tile([128, 8, 128], bf16, tag=f"ctx{k}",
                                     name=f"ctx{k}")
                eng.dma_start(ctx_sb[:],
                              a2a_out[k][:].rearrange("d p c -> p d c"))
                ctx_tiles[k] = ctx_sb

            def outproj_quarter(k, store_eng, split_store=False):
                ctx_sb = ctx_tiles[k]
                ot = outp.tile([128, 1024], bf16, tag="ot")
                for oh in range(2):      # 512-wide od halves (PSUM bank limit)
                    ps = ps_misc.tile([128, TT], f32, tag="mm")
                    for d in range(8):
                        nc.tensor.matmul(
                            ps[:], ctx_sb[:, d, :],
                            wo_sb[:, d, TT * oh:TT * (oh + 1)],
                            start=(d == 0), stop=(d == 7))
                    nc.vector.tensor_tensor(
                        ot[:, TT * oh:TT * (oh + 1)], ps[:],
                        bo_sb[:, TT * oh:TT * (oh + 1)],
                        mybir.AluOpType.add)
                    if split_store:  # ship each half as soon as its add lands
                        store_eng.dma_start(
                            outT_d[k, :, TT * oh:TT * (oh + 1)],
                            ot[:, TT * oh:TT * (oh + 1)])
                if not split_store:
                    store_eng.dma_start(outT_d[k], ot[:])

            # ---- pipelined schedule. Projection pairs run ahead of their
            # consumers; norm(q) finishes at iteration q+2 (a full iteration
            # of slack, so its PE outer-product never waits on the reciprocal
            # chain); collective triggers (gpsimd ring) fire as soon as both
            # contributing ships are in.
            proj_pair(0)                     # tiles 0,1 (b0 j0/j1)
            norms = {}
            for i in range(NT):
                if i == 0:
                    proj_pair(2)             # tiles 4,5 (b1 j0/j1)
                elif i == 2:
                    proj_pair(1)             # tiles 2,3 (b0 j2/j3)
                elif i == 3:
                    proj_pair(3)             # tiles 6,7 (b1 j2/j3)
                if i >= 2:
                    finish_norm(norms.pop(i - 2))
                if i == 6:
                    load_ctx(0, nc.sync)     # a2a(0) done an iteration ago
                elif i == 7:
                    # G2 = both j=2 tiles; norm(5) just finished (i>=2 rule),
                    # so a2a(2) flies DURING the last (longest) attention
                    # tile and ctx(2) lands mid-i7
                    do_a2a(2)                # ships of q-tiles 4 (i6) + 5 (i7)
                    finish_norm(norms.pop(6))
                    load_ctx(1, nc.sync)     # a2a(1) done an iteration ago
                    load_ctx(2, nc.sync)
                if i == 4:
                    do_a2a(0)                # ships of q-tiles 0 (i2) + 2 (i4)
                elif i == 5:
                    do_a2a(1)                # ships of q-tiles 1 (i3) + 3 (i5)
                vtrans_tile(TILE_OF[i])
                norms[i] = attention_qtile(*ORDER[i], last=(i == NT - 1))
            # quarters 0-1 are emitted BEFORE the last norm (no semaphore-
            # counter dep on its Pool broadcasts -> they fill the PE right
            # after the last PV, including on the straggler core); their
            # stores ride the Act ring so ships(7) aren't queued behind
            # them on Sync. Quarter 2 (emitted after the trigger) fills the
            # PE while a2a(3) flies.
            outproj_quarter(0, nc.scalar)
            outproj_quarter(1, nc.scalar)
            finish_norm_last(norms.pop(7))
            do_a2a(3)                        # ships of q-tiles 6 (i6) + 7
            outproj_quarter(2, nc.sync)
            load_ctx(3, nc.scalar)
            outproj_quarter(3, nc.scalar, split_store=True)

    nc.compile()
    return nc


def make_in_maps(x, Wq, bq, Wk, bk, Wv, bv, Wo, bo):
    x = np.asarray(x, np.float32)
    xT = np.ascontiguousarray(x.reshape(T, D).T)                  # [D, T]
    # [NP, 128, 8, PT]: xTp[p, part, o, q] = xT[o*128+part, p*PT+q]
    xTp = np.ascontiguousarray(
        xT.reshape(8, 128, NP, PT).transpose(2, 1, 0, 3)).astype(npbf)

    woT = np.ascontiguousarray(
        np.asarray(Wo, np.float32).T.reshape(8, 128, 1024)
        .transpose(1, 0, 2)).astype(npbf)
    bo_row = np.ascontiguousarray(np.asarray(bo, np.float32)[None, :])

    trimask = (np.arange(128)[:, None] <= np.arange(128)[None, :]).astype(npbf)
    ident = np.eye(128, dtype=npbf)

    in_maps = []
    for c in range(NCORE):
        sl = slice(128 * c, 128 * (c + 1))
        wT_c = np.stack(
            [np.ascontiguousarray(
                np.asarray(W, np.float32)[sl, :].T.reshape(8, 128, 128)
                .transpose(1, 0, 2))
             for W in (Wq, Wk, Wv)], axis=2)                       # [128, 8, 3, 128]
        bqkv_c = np.stack([np.asarray(b_, np.float32)[sl]
                           for b_ in (bq, bk, bv)], axis=1)        # [128, 3]
        # dict order ~ restage order, by first consumption (see the
        # dram_tensor declarations)
        in_maps.append({
            "xTp0": np.ascontiguousarray(xTp[0]),
            "wT": np.ascontiguousarray(wT_c).astype(npbf),
            "bqkv": np.ascontiguousarray(bqkv_c),
            "xTp2": np.ascontiguousarray(xTp[2]),
            "ident": ident,
            "trimask": trimask,
            "xTp1": np.ascontiguousarray(xTp[1]),
            "xTp3": np.ascontiguousarray(xTp[3]),
            "bo": bo_row,
            "woT": woT,
        })
    return in_maps


def group_token(k, c):
    """(batch, seq start) of core c's 128-token slice of a2a group k."""
    if k < 2:
        return k, 128 * c
    return c // 4, 512 * k + 128 * (c % 4)


def assemble_output(results):
    # results[c]["outT"]: [4, 128, 1024]; slice k covers group_token(k, c)
    out = np.empty((B, S, D), np.float32)
    for c in range(NCORE):
        for k in range(4):
            b, t0 = group_token(k, c)
            out[b, t0:t0 + 128, :] = np.asarray(results[c]["outT"][k], np.float32)
    return out


_PROGRAM = None


def get_program():
    global _PROGRAM
    if _PROGRAM is None:
        _PROGRAM = build_program()
    return _PROGRAM


def run(in_maps, **kwargs):
    nc = get_program()
    return run_bass_kernel_spmd(nc, in_maps, core_ids=list(range(NCORE)), **kwargs)


def kernel(x, Wq, bq, Wk, bk, Wv, bv, Wo, bo):
    in_maps = make_in_maps(x, Wq, bq, Wk, bk, Wv, bv, Wo, bo)
    res = run(in_maps)
    return assemble_output(res.results)


if __name__ == "__main__":
    rng = np.random.default_rng(0)
    x = rng.standard_normal((B, S, D), dtype=np.float32)
    mk = lambda *s: ((rng.random(s).astype(np.float32)) - 0.5) / 16
    out = kernel(x, mk(D, D), mk(D), mk(D, D), mk(D), mk(D, D), mk(D),
                 mk(D, D), mk(D))
    print(out.shape, out.dtype, np.abs(out).mean())



# revision 46
# speedup vs baseline: 1.1079x; 1.0297x over previous
"""Multi-head causal attention (B=2, S=2048, D=1024, H=16) on 8 trn2 NeuronCores.

Strategy (tensor-parallel over heads, per the sharding hint):
  - Each core owns 2 heads (128 of 1024 hidden dims): W_q/W_k/W_v column-parallel.
  - Activations kept transposed ([dim, token]) end to end so every matmul
    contracts on the partition axis with zero on-device transposes of x.
  - Projections run per 1024-token pair of tiles (x loaded in 2MB chunks);
    each matmul streams 512 tokens (one fp32 PSUM bank); attention q-tiles
    are 512 wide. Diagonal-chunk score/PV matmuls stream only the unmasked
    query range.
  - scores^T = K^T.T @ Q^T per 128-key-chunk x 512-query-tile, two heads packed
    into disjoint PE row-groups (contraction is only dk=64).
  - softmax without max-subtraction (scores are O(1)); rowsum folded into the
    PV matmul via an augmented V [keys, 64+1] whose last column is ones.
  - exp only on the causal part of diagonal chunks; the rest of the P tile is
    zeroed, and only the 128-wide diagonal strip is tri-masked.
  - normalization: rowsum rows gathered to [128, 8] for one 128-lane DVE
    reciprocal, scattered back, broadcast via a PE outer-product; the finish
    (broadcast+multiply+ship) is deferred >= one full iteration so the PE
    never waits on the chain.
  - q-tiles processed batch-interleaved (b0j0, b1j0, b0j1, ...) and ctx
    re-sharded token-parallel with FOUR AllToAlls (one per half-batch); the
    gpsimd ring carries ONLY the collective triggers, so a busy CC engine
    can never stall compute; each a2a DRAM buffer has its own pool tag
    (shared-tag tiles alias one slot and serialize ships behind collectives).
  - out-projection (full W_o) per 128-token quarter at the tail, filling
    the PE while a2a(2)/(3) fly; only the last a2a + one quarter is exposed.
  - bf16 matmul inputs everywhere; PSUM accumulation stays fp32; the
    softmax reciprocal and the final output are bf16.

kernel(**inputs) takes the full unsharded inputs and returns the full output.
"""

import numpy as np
import ml_dtypes

import concourse.bass as bass
import concourse.mybir as mybir
import concourse.tile as tile
from concourse import bacc
from concourse.bass_utils import run_bass_kernel_spmd

B, S, D = 2, 2048, 1024
H, DK = 16, 64
NCORE = 8
T = B * S          # 4096 tokens
TT = 512           # attention q-tile width
PT = 1024          # projection pair width
NT = T // TT       # 8 token tiles
NP = T // PT       # 4 projection pairs
KC = 128           # key chunk
NJ = S // TT       # 4 q-tiles per batch
SCALE = 1.0 / np.sqrt(DK)

# batch-interleaved q-tile order; ORDER[i] = (b, j), its token tile is b*NJ+j
ORDER = [(0, 0), (1, 0), (0, 1), (1, 1), (0, 2), (1, 2), (0, 3), (1, 3)]
TILE_OF = [b * NJ + j for (b, j) in ORDER]
# a2a group of q-tile (b, j); groups pair tiles that finish adjacently so
# each a2a's inputs complete as early as possible: G0/G1 = j<2 per batch,
# G2 = both j=2 tiles (done by i5), G3 = both j=3 tiles (the tail pair)
A_OF = {(b, j): (b if j < 2 else j) for (b, j) in ORDER}
# dst slab base within a group: j<2 -> by j, j>=2 -> by batch
G0_OF = {(b, j): (4 * j if j < 2 else 4 * b) for (b, j) in ORDER}

f32 = mybir.dt.float32
bf16 = mybir.dt.bfloat16
EXP = mybir.ActivationFunctionType.Exp
MULT = mybir.AluOpType.mult
npbf = ml_dtypes.bfloat16


def build_program():
    nc = bacc.Bacc("TRN2", target_bir_lowering=False, debug=False,
                   num_devices=NCORE)

    # declaration order ~ host restage order: xTp0 + wT first (they gate the
    # first projection), xTp1-3 stream behind, woT last (tail-only)
    # restage follows declaration order; order by first consumption:
    # xTp0 + wT + bqkv gate the first projection, xTp2 is consumed at i=0,
    # ident/trimask at the first vtrans/diag chunk, xTp1 at i=2, xTp3 at
    # i=3, woT only at the tail
    def xin(name, shape):
        return nc.dram_tensor(name, shape, bf16, kind="ExternalInput").ap()

    xTp_d = [None] * NP
    xTp_d[0] = xin("xTp0", [128, 8, PT])
    wT_d = xin("wT", [128, 8, 3, 128])
    bqkv_d = nc.dram_tensor("bqkv", [128, 3], f32, kind="ExternalInput").ap()
    xTp_d[2] = xin("xTp2", [128, 8, PT])
    ident_d = xin("ident", [128, 128])
    trimask_d = xin("trimask", [128, 128])
    xTp_d[1] = xin("xTp1", [128, 8, PT])
    xTp_d[3] = xin("xTp3", [128, 8, PT])
    bo_d = nc.dram_tensor("bo", [1, 1024], f32, kind="ExternalInput").ap()
    woT_d = xin("woT", [128, 8, 1024])
    # outT[k] = this core's 128-token slice of a2a group k (see GROUP_TOK)
    outT_d = nc.dram_tensor("outT", [4, 128, 1024], bf16, kind="ExternalOutput").ap()

    with tile.TileContext(nc) as tc:
        with (
            tc.tile_pool(name="const", bufs=1) as constp,
            tc.tile_pool(name="wostream", bufs=1) as wop,
            tc.tile_pool(name="xstream", bufs=2) as xp,
            tc.tile_pool(name="qkv", bufs=NP) as qkvp,
            tc.tile_pool(name="vaug", bufs=NJ) as vaugp,
            tc.tile_pool(name="ptile", bufs=4) as pp,
            tc.tile_pool(name="post", bufs=2) as postp,
            tc.tile_pool(name="cxn", bufs=2) as cxnp,
            tc.tile_pool(name="outsb", bufs=2) as outp,
            tc.tile_pool(name="ps_s", bufs=2, space="PSUM") as ps_s,
            tc.tile_pool(name="ps_ctx", bufs=1, space="PSUM") as ps_ctx,
            tc.tile_pool(name="ps_misc", bufs=2, space="PSUM") as ps_misc,
            tc.tile_pool(name="dram", bufs=1, space="DRAM") as dramp,
        ):
            # ---- constants; x pair 0 split per-chunk unblocks the PE early.
            # x rides the Scalar ring, consts the Sync ring (parallel rings).
            # x pair 0 rides the Act ring (idle at startup) so its issues run
            # in parallel with the consts on the Sync ring
            xt0 = xp.tile([128, 8, PT], bf16, tag="xt")
            for o in range(8):      # 8 x 256KB: finer grains land earlier
                nc.scalar.dma_start(xt0[:, o, :], xTp_d[0][:, o, :])
            wT = constp.tile([128, 8, 3, 128], bf16, tag="wT")
            nc.sync.dma_start(wT[:], wT_d)
            ident = constp.tile([128, 128], bf16, tag="ident")
            nc.sync.dma_start(ident[:], ident_d)
            bqkv = constp.tile([128, 3], f32, tag="bqkv")
            nc.sync.dma_start(bqkv[:], bqkv_d)
            trimask = constp.tile([128, 128], bf16, tag="trimask")
            nc.sync.dma_start(trimask[:], trimask_d)

            # W_o / b_o ride the gpsimd ring once, before any collectives
            wo_sb = wop.tile([128, 8, 1024], bf16, tag="wo")
            nc.gpsimd.dma_start(wo_sb[:], woT_d)
            bo_row = wop.tile([1, 1024], f32, tag="bor")
            nc.gpsimd.dma_start(bo_row[:], bo_d)
            bo_sb = wop.tile([128, 1024], f32, tag="bobc")
            nc.gpsimd.partition_broadcast(bo_sb[:], bo_row[:], channels=128)

            # per-pair Q/K/V (transposed, [128, 1024]) and per-tile augmented V
            qkv_t = [[None] * NP for _ in range(3)]
            vaug_t = [[None] * NJ for _ in range(B)]

            # four a2a groups; dst core c <- its 128-token slice of each group.
            # DISTINCT tags: same-tag pool tiles alias one ring slot, which
            # would serialize ships of group k+1 behind the collective read
            # of group k.
            a2a_in = [dramp.tile([NCORE, 128, 128], bf16, name=f"a2a_in{k}",
                                 tag=f"a2a_in{k}")
                      for k in range(4)]
            a2a_out = [dramp.tile([NCORE, 128, 128], bf16, name=f"a2a_out{k}",
                                  tag=f"a2a_out{k}")
                       for k in range(4)]

            def proj_pair(p):
                if p == 0:
                    xt = xt0
                else:
                    xt = xp.tile([128, 8, PT], bf16, tag="xt")
                    for g in range(2):   # 2 x 1MB halves
                        nc.sync.dma_start(xt[:, 4 * g:4 * (g + 1), :],
                                          xTp_d[p][:, 4 * g:4 * (g + 1), :])
                for j in range(3):
                    qt = qkvp.tile([128, PT], bf16, tag=f"qkv{j}",
                                   name=f"qkv{j}_{p}")
                    # one matmul may write at most one 2KB PSUM bank (512
                    # fp32), so each 1024-token pair projects in two halves
                    for half in range(2):
                        ps = ps_misc.tile([128, TT], f32, tag="mm")
                        for o in range(8):
                            nc.tensor.matmul(
                                ps[:], wT[:, o, j, :],
                                xt[:, o, half * TT:(half + 1) * TT],
                                start=(o == 0), stop=(o == 7))
                        nc.vector.tensor_scalar_add(
                            qt[:, half * TT:(half + 1) * TT], ps[:],
                            bqkv[:, j:j + 1])
                    qkv_t[j][p] = qt

            def qslice(j, t, lo, hi):
                return qkv_t[j][t // 2][:, (t % 2) * TT + lo:(t % 2) * TT + hi]

            def vtrans_tile(t):
                b, tl = t // NJ, t % NJ
                # both heads in one tile, each head's ones-column at the END
                # of its 65-wide block, so one strided copy fills both heads
                va = vaugp.tile([128, NJ, 2 * (DK + 1)], bf16, tag=f"va{b}",
                                name=f"va{b}_{tl}")
                nc.vector.memset(
                    va[:].rearrange("p k (g c) -> p k g c", g=2)[:, :, :, DK:DK + 1],
                    1.0)
                vaug_t[b][tl] = va
                for kt in range(NJ):
                    ps_t = ps_misc.tile([128, TT], bf16, tag="mm")
                    nc.tensor.transpose(ps_t[:, 0:128],
                                        qslice(2, t, kt * KC, (kt + 1) * KC),
                                        ident[:])
                    nc.vector.tensor_copy(
                        va[:, kt, :].rearrange("p (g c) -> p g c", g=2)[:, :, 0:DK],
                        ps_t[:, 0:128].rearrange("p (g c) -> p g c", g=2))

            def attention_qtile(b, j, mid_hook=None, last=False):
                nk = 4 * (j + 1)
                pc = [ps_ctx.tile([DK + 1, TT], f32, tag=f"c{h}", name=f"pc{h}")
                      for h in range(2)]

                def emit_pv(p_tile, m):
                    # the masked query range of a diagonal chunk is all-zero
                    # P - skip streaming it (m == 0 is always full range)
                    q0 = max(m - 4 * j, 0) * KC
                    for h in range(2):
                        nc.tensor.matmul(
                            pc[h][:, q0:],
                            vaug_t[b][m // 4][:, m % 4,
                                              (DK + 1) * h:(DK + 1) * (h + 1)],
                            p_tile[:, TT * h + q0:TT * (h + 1)],
                            start=(m == 0), stop=(m == nk - 1),
                            skip_group_check=True)

                pending = []
                for m in range(nk):
                    tk = b * NJ + m // 4
                    ko = (m % 4) * KC
                    # queries below the diagonal chunk's start are masked out
                    # anyway - don't stream them through the PE
                    q0 = max(m - 4 * j, 0) * KC
                    ps = ps_s.tile([128, 2 * TT], f32, tag="s")
                    nc.tensor.matmul(ps[:, q0:TT],
                                     qslice(1, tk, ko, ko + KC)[0:DK, :],
                                     qslice(0, b * NJ + j, q0, TT)[0:DK, :],
                                     start=True, stop=True, tile_position=(0, 0))
                    nc.tensor.matmul(ps[:, TT + q0:],
                                     qslice(1, tk, ko, ko + KC)[DK:128, :],
                                     qslice(0, b * NJ + j, q0, TT)[DK:128, :],
                                     start=True, stop=True, tile_position=(64, 0))
                    p = pp.tile([128, 2 * TT], bf16, tag="p")
                    r = m - 4 * j
                    if r >= 0:
                        # cols [0, KC*r) are never streamed by emit_pv (its
                        # q0 skips them), so they need no zeroing
                        nc.scalar.activation(
                            p[:].rearrange("k (h q) -> k h q", h=2)[:, :, KC * r:],
                            ps[:].rearrange("k (h q) -> k h q", h=2)[:, :, KC * r:],
                            EXP, scale=float(SCALE))
                        nc.vector.tensor_tensor(
                            p[:].rearrange("k (h q) -> k h q", h=2)[:, :, KC * r:KC * (r + 1)],
                            p[:].rearrange("k (h q) -> k h q", h=2)[:, :, KC * r:KC * (r + 1)],
                            trimask[:, None, :].to_broadcast([128, 2, 128]), MULT)
                    else:
                        nc.scalar.activation(p[:], ps[:], EXP, scale=float(SCALE))
                    pending.append((p, m))
                    if len(pending) > 2:   # depth-2: PE never waits on a fresh exp
                        emit_pv(*pending.pop(0))
                    if m == 3 and mid_hook is not None:
                        mid_hook()   # e.g. late norm finish + a2a trigger
                for pm in pending:
                    emit_pv(*pm)

                if last:
                    # the final q-tile's norm isn't deferred and nothing
                    # recycles its PSUM banks: finish_norm_last reads pc
                    # directly (no cx copy, no DVE reciprocal chain)
                    return {"pc": pc, "b": b, "j": j}

                # normalization phase 1 (phase 2 deferred via finish_norm):
                # the rowsum row is spread across 32 DVE lanes via a block-
                # transpose, reciprocal'd batched ([32, 16] view instead of a
                # 1-lane [1, 512] at ~3.3us), and transposed back - all on
                # the DVE, so no DMA ever races collective channel traffic.
                cxs, rrows = [], []
                for h in range(2):
                    rt = cxnp.tile([32, TT], f32, tag="rt")
                    nc.vector.tensor_copy(rt[0:1, :], pc[h][DK:DK + 1, :])
                    # cx lives until finish_norm two iterations later, so two
                    # q-tiles' worth of cx tiles (2 heads each) coexist
                    cx = cxnp.tile([DK, TT], f32, tag="cx", bufs=4)
                    nc.vector.tensor_copy(cx[:], pc[h][0:DK, :])
                    cxs.append(cx)
                    rtT = cxnp.tile([32, TT], f32, tag="rtT")
                    nc.vector.transpose(rtT[:], rt[:])
                    rcT = cxnp.tile([32, TT], bf16, tag="rcT")
                    with nc.allow_low_precision(reason="softmax denominator"):
                        nc.vector.reciprocal(
                            rcT[:].rearrange("p (b c) -> p b c", c=32)[:, :, 0:1],
                            rtT[:].rearrange("p (b c) -> p b c", c=32)[:, :, 0:1])
                    rrow = cxnp.tile([32, TT], bf16, tag="rrow", bufs=4)
                    nc.vector.transpose(rrow[:], rcT[:])
                    rrows.append(rrow)   # row 0 = per-query reciprocal
                return {"cxs": cxs, "rrows": rrows, "b": b, "j": j}

            def finish_norm(st):
                # phase 2: Pool broadcasts each head's reciprocal row across
                # its 64 partitions (keeps the PE out of the norm chain),
                # then one multiply per head and one 3D-pattern ship per head
                # (dma_start issue is the scarce resource: ~0.63us each
                # through the shared HWDGE).
                b, j = st["b"], st["j"]
                k = A_OF[(b, j)]
                bcs = []
                for h in range(2):
                    bc = cxnp.tile([DK, TT], bf16, tag=f"bc{h}")
                    nc.gpsimd.partition_broadcast(
                        bc[:], st["rrows"][h][0:1, :], channels=DK)
                    bcs.append(bc)
                g0 = G0_OF[(b, j)]
                for h in range(2):
                    cxn = cxnp.tile([DK, TT], bf16, tag="cxn")
                    nc.vector.tensor_tensor(cxn[:], st["cxs"][h][:],
                                            bcs[h][:], MULT)
                    nc.sync.dma_start(
                        a2a_in[k][g0:g0 + 4, DK * h:DK * (h + 1), :]
                        .rearrange("g p c -> p g c"),
                        cxn[:].rearrange("p (g c) -> p g c", g=4))

            def finish_norm_last(st):
                # tail-latency path for the final q-tile: Act (idle after the
                # last exp) reciprocals the rowsum rows straight out of PSUM,
                # Pool broadcasts them, and the DVE multiply reads the
                # context numerator directly from PSUM - ships fire ~8us
                # sooner than the deferred-norm chain would manage.
                b, j = st["b"], st["j"]
                k, g0 = A_OF[(b, j)], G0_OF[(b, j)]
                pc = st["pc"]
                for h in range(2):
                    rt = cxnp.tile([32, TT], f32, tag="rt")
                    nc.vector.tensor_copy(rt[0:1, :], pc[h][DK:DK + 1, :])
                    rtT = cxnp.tile([32, TT], f32, tag="rtT")
                    nc.vector.transpose(rtT[:], rt[:])
                    rcT = cxnp.tile([32, TT], bf16, tag="rcT")
                    with nc.allow_low_precision(reason="softmax denominator"):
                        nc.vector.reciprocal(
                            rcT[:].rearrange("p (b c) -> p b c", c=32)[:, :, 0:1],
                            rtT[:].rearrange("p (b c) -> p b c", c=32)[:, :, 0:1])
                    rrow = cxnp.tile([32, TT], bf16, tag=f"rr{h}")
                    nc.vector.transpose(rrow[:], rcT[:])
                    bc = cxnp.tile([DK, TT], bf16, tag=f"bc{h}")
                    nc.gpsimd.partition_broadcast(bc[:], rrow[0:1, :],
                                                  channels=DK)
                    cxn = cxnp.tile([DK, TT], bf16, tag="cxn")
                    nc.vector.tensor_tensor(cxn[:], pc[h][0:DK, :], bc[:], MULT)
                    nc.sync.dma_start(
                        a2a_in[k][g0:g0 + 4, DK * h:DK * (h + 1), :]
                        .rearrange("g p c -> p g c"),
                        cxn[:].rearrange("p (g c) -> p g c", g=4))

            def do_a2a(k):
                nc.gpsimd.collective_compute(
                    "AllToAll", mybir.AluOpType.bypass,
                    replica_groups=[list(range(NCORE))],
                    ins=[a2a_in[k][:].opt()], outs=[a2a_out[k][:].opt()])

            ctx_tiles = {}

            def load_ctx(k, eng):
                # ctx(0)/(1) prefetch mid-kernel on the Sync ring (their
                # collectives are long done, and transfers avoid the tail's
                # collective channel traffic); ctx(2)/(3) load at the tail
                # on the Scalar ring (exps finished, Sync stays clear for
                # ships(7) -> a2a(3))
                ctx_sb = constp.tile([128, 8, 128], bf16, tag=f"ctx{k}",
                                     name=f"ctx{k}")
                eng.dma_start(ctx_sb[:],
                              a2a_out[k][:].rearrange("d p c -> p d c"))
                ctx_tiles[k] = ctx_sb

            def outproj_quarter(k, store_eng, split_store=False):
                ctx_sb = ctx_tiles[k]
                ot = outp.tile([128, 1024], bf16, tag="ot")
                for oh in range(2):      # 512-wide od halves (PSUM bank limit)
                    ps = ps_misc.tile([128, TT], f32, tag="mm")
                    for d in range(8):
                        nc.tensor.matmul(
                            ps[:], ctx_sb[:, d, :],
                            wo_sb[:, d, TT * oh:TT * (oh + 1)],
                            start=(d == 0), stop=(d == 7))
                    nc.vector.tensor_tensor(
                        ot[:, TT * oh:TT * (oh + 1)], ps[:],
                        bo_sb[:, TT * oh:TT * (oh + 1)],
                        mybir.AluOpType.add)
                    if split_store:  # ship each half as soon as its add lands
                        store_eng.dma_start(
                            outT_d[k, :, TT * oh:TT * (oh + 1)],
                            ot[:, TT * oh:TT * (oh + 1)])
                if not split_store:
                    store_eng.dma_start(outT_d[k], ot[:])

            # ---- pipelined schedule. Projection pairs run ahead of their
            # consumers; norm(q) finishes at iteration q+2 (a full iteration
            # of slack, so its PE outer-product never waits on the reciprocal
            # chain); collective triggers (gpsimd ring) fire as soon as both
            # contributing ships are in.
            proj_pair(0)                     # tiles 0,1 (b0 j0/j1)
            norms = {}
            for i in range(NT):
                if i == 0:
                    proj_pair(2)             # tiles 4,5 (b1 j0/j1)
                elif i == 2:
                    proj_pair(1)             # tiles 2,3 (b0 j2/j3)
                elif i == 3:
                    proj_pair(3)             # tiles 6,7 (b1 j2/j3)
                if i >= 2:
                    finish_norm(norms.pop(i - 2))
                if i == 6:
                    load_ctx(0, nc.sync)     # a2a(0) done an iteration ago
                elif i == 7:
                    # G2 = both j=2 tiles; norm(5) just finished (i>=2 rule),
                    # so a2a(2) flies DURING the last (longest) attention
                    # tile and ctx(2) lands mid-i7
                    do_a2a(2)                # ships of q-tiles 4 (i6) + 5 (i7)
                    finish_norm(norms.pop(6))
                    load_ctx(1, nc.sync)     # a2a(1) done an iteration ago
                    # ctx(2) on the Act ring: its wait for a2a(2) (peer-
                    # gated) must not block ships(7) behind it on Sync
                    load_ctx(2, nc.scalar)
                if i == 4:
                    do_a2a(0)                # ships of q-tiles 0 (i2) + 2 (i4)
                elif i == 5:
                    do_a2a(1)                # ships of q-tiles 1 (i3) + 3 (i5)
                vtrans_tile(TILE_OF[i])
                norms[i] = attention_qtile(*ORDER[i], last=(i == NT - 1))
            # quarters 0-1 are emitted BEFORE the last norm (no semaphore-
            # counter dep on its Pool broadcasts -> they fill the PE right
            # after the last PV, including on the straggler core); their
            # stores ride the Act ring so ships(7) aren't queued behind
            # them on Sync. Quarter 2 (emitted after the trigger) fills the
            # PE while a2a(3) flies.
            outproj_quarter(0, nc.scalar)
            outproj_quarter(1, nc.scalar)
            finish_norm_last(norms.pop(7))
            do_a2a(3)                        # ships of q-tiles 6 (i6) + 7
            outproj_quarter(2, nc.sync)
            load_ctx(3, nc.scalar)
            outproj_quarter(3, nc.scalar, split_store=True)

    nc.compile()
    return nc


def make_in_maps(x, Wq, bq, Wk, bk, Wv, bv, Wo, bo):
    x = np.asarray(x, np.float32)
    xT = np.ascontiguousarray(x.reshape(T, D).T)                  # [D, T]
    # [NP, 128, 8, PT]: xTp[p, part, o, q] = xT[o*128+part, p*PT+q]
    xTp = np.ascontiguousarray(
        xT.reshape(8, 128, NP, PT).transpose(2, 1, 0, 3)).astype(npbf)

    woT = np.ascontiguousarray(
        np.asarray(Wo, np.float32).T.reshape(8, 128, 1024)
        .transpose(1, 0, 2)).astype(npbf)
    bo_row = np.ascontiguousarray(np.asarray(bo, np.float32)[None, :])

    trimask = (np.arange(128)[:, None] <= np.arange(128)[None, :]).astype(npbf)
    ident = np.eye(128, dtype=npbf)

    in_maps = []
    for c in range(NCORE):
        sl = slice(128 * c, 128 * (c + 1))
        wT_c = np.stack(
            [np.ascontiguousarray(
                np.asarray(W, np.float32)[sl, :].T.reshape(8, 128, 128)
                .transpose(1, 0, 2))
             for W in (Wq, Wk, Wv)], axis=2)                       # [128, 8, 3, 128]
        bqkv_c = np.stack([np.asarray(b_, np.float32)[sl]
                           for b_ in (bq, bk, bv)], axis=1)        # [128, 3]
        # dict order ~ restage order, by first consumption (see the
        # dram_tensor declarations)
        in_maps.append({
            "xTp0": np.ascontiguousarray(xTp[0]),
            "wT": np.ascontiguousarray(wT_c).astype(npbf),
            "bqkv": np.ascontiguousarray(bqkv_c),
            "xTp2": np.ascontiguousarray(xTp[2]),
            "ident": ident,
            "trimask": trimask,
            "xTp1": np.ascontiguousarray(xTp[1]),
            "xTp3": np.ascontiguousarray(xTp[3]),
            "bo": bo_row,
            "woT": woT,
        })
    return in_maps


def group_token(k, c):
    """(batch, seq start) of core c's 128-token slice of a2a group k."""
    if k < 2:
        return k, 128 * c
    return c // 4, 512 * k + 128 * (c % 4)


def assemble_output(results):
    # results[c]["outT"]: [4, 128, 1024]; slice k covers group_token(k, c)
    out = np.empty((B, S, D), np.float32)
    for c in range(NCORE):
        for k in range(4):
            b, t0 = group_token(k, c)
            out[b, t0:t0 + 128, :] = np.asarray(results[c]["outT"][k], np.float32)
    return out


_PROGRAM = None


def get_program():
    global _PROGRAM
    if _PROGRAM is None:
        _PROGRAM = build_program()
    return _PROGRAM


def run(in_maps, **kwargs):
    nc = get_program()
    return run_bass_kernel_spmd(nc, in_maps, core_ids=list(range(NCORE)), **kwargs)


def kernel(x, Wq, bq, Wk, bk, Wv, bv, Wo, bo):
    in_maps = make_in_maps(x, Wq, bq, Wk, bk, Wv, bv, Wo, bo)
    res = run(in_maps)
    return assemble_output(res.results)


if __name__ == "__main__":
    rng = np.random.default_rng(0)
    x = rng.standard_normal((B, S, D), dtype=np.float32)
    mk = lambda *s: ((rng.random(s).astype(np.float32)) - 0.5) / 16
    out = kernel(x, mk(D, D), mk(D), mk(D, D), mk(D), mk(D, D), mk(D),
                 mk(D, D), mk(D))
    print(out.shape, out.dtype, np.abs(out).mean())

